# revision 1
# baseline (speedup 1.0000x reference)
import numpy as np

import concourse.bass as bass
import concourse.bacc as bacc
import concourse.mybir as mybir
import concourse.tile as tile
from concourse.bass_utils import run_bass_kernel_spmd
from concourse.masks import make_identity

FP = mybir.dt.float32
FR = mybir.dt.float32r
BF = mybir.dt.bfloat16
AF = mybir.ActivationFunctionType
OP = mybir.AluOpType

GRID = 32
NN = 1024
F_IN = 32
H = 256
B = 64
S = 8
NCORE = 8
NB = 8
OBS_W = NN + NN * F_IN
MIN_VAL = -10000000.0
EPS_LN = 1e-5
EPS_BN = 1e-5
PAD = 32
HW = NN + 2 * PAD

USE_BF16 = False
PROFILE = False
LAST_EXEC_NS = None
TRACE_KWARGS = {}


def _build(has_gin_bias: bool, b2_val: float, use_bf16: bool) -> bass.Bass:
    nc = bacc.Bacc("TRN2", target_bir_lowering=False, debug=False)

    MT = BF if use_bf16 else FP
    GI = mybir.dt.uint16 if use_bf16 else mybir.dt.uint32

    def mm(ap):
        return ap if use_bf16 else ap.bitcast(FR)

    obs = nc.declare_dram_parameter("obs", [S, OBS_W], FP, isOutput=False)
    w0 = nc.declare_dram_parameter("w0", [F_IN, H], FP, isOutput=False)
    ws = nc.declare_dram_parameter("ws", [3, 2, 128, H], FP, isOutput=False)
    w1x = nc.declare_dram_parameter("w1x", [F_IN, 512], FP, isOutput=False)
    w1h = nc.declare_dram_parameter("w1h", [8, 128, 512], FP, isOutput=False)
    w2 = nc.declare_dram_parameter("w2", [4, 128], FP, isOutput=False)
    gg = nc.declare_dram_parameter("gg", [4, H], FP, isOutput=False)
    bb = nc.declare_dram_parameter("bb", [4, H], FP, isOutput=False)
    bns = nc.declare_dram_parameter("bns", [512], FP, isOutput=False)
    bnt = nc.declare_dram_parameter("bnt", [512], FP, isOutput=False)
    if has_gin_bias:
        gbias = nc.declare_dram_parameter("gbias", [4, H], FP, isOutput=False)
    y_out = nc.declare_dram_parameter("y", [S, NN], FP, isOutput=True)

    from contextlib import ExitStack

    with tile.TileContext(nc) as tc, ExitStack() as ctx:
        wp = ctx.enter_context(tc.tile_pool(name="w", bufs=1))
        px = ctx.enter_context(tc.tile_pool(name="px", bufs=2))
        ph = ctx.enter_context(tc.tile_pool(name="ph", bufs=2))
        pst = ctx.enter_context(tc.tile_pool(name="pst", bufs=8))
        pfin = ctx.enter_context(tc.tile_pool(name="pfin", bufs=1))
        pz = ctx.enter_context(tc.tile_pool(name="pz", bufs=3, space="PSUM"))
        ptf = ctx.enter_context(tc.tile_pool(name="ptf", bufs=5, space="PSUM"))

        ident = wp.tile([128, 128], MT, tag="id")
        make_identity(nc, ident[:])
        eps_sb = wp.tile([128, 1], FP, tag="eps")
        nc.gpsimd.memset(eps_sb[:], EPS_LN)

        w0_sb = wp.tile([F_IN, H], MT, tag="w0")
        nc.gpsimd.dma_start(mm(w0_sb[:]), w0[:, :])

        wl_sb = []
        for l in range(3):
            t = wp.tile([128, 2 * H], MT, tag=f"wl{l}")
            nc.gpsimd.dma_start(
                mm(t[:]).rearrange("p (k n) -> p k n", k=2),
                ws[l].rearrange("k p n -> p k n"),
            )
            wl_sb.append(t)

        w1x_sb = wp.tile([F_IN, 512], MT, tag="w1x")
        nc.gpsimd.dma_start(mm(w1x_sb[:]), w1x[:, :])
        w1h_sb = wp.tile([128, 8 * 512], MT, tag="w1h")
        nc.gpsimd.dma_start(
            mm(w1h_sb[:]).rearrange("p (j m) -> p j m", j=8),
            w1h[:, :, :].rearrange("j p m -> p j m"),
        )
        w2_sb = wp.tile([128, 4], MT, tag="w2")
        nc.gpsimd.dma_start(mm(w2_sb[:]), w2[:, :].rearrange("k p -> p k"))

        gg_sb = wp.tile([128, 8], FP, tag="gg")
        nc.sync.dma_start(
            gg_sb[:].rearrange("p (l c) -> p l c", c=2),
            gg[:, :].rearrange("l (c p) -> p l c", p=128),
        )
        bb_sb = wp.tile([128, 8], FP, tag="bb")
        nc.sync.dma_start(
            bb_sb[:].rearrange("p (l c) -> p l c", c=2),
            bb[:, :].rearrange("l (c p) -> p l c", p=128),
        )
        bns_sb = wp.tile([128, 4], FP, tag="bns")
        nc.sync.dma_start(bns_sb[:], bns[:].rearrange("(m p) -> p m", p=128))
        bnt_sb = wp.tile([128, 4], FP, tag="bnt")
        nc.sync.dma_start(bnt_sb[:], bnt[:].rearrange("(m p) -> p m", p=128))

        if has_gin_bias:
            ones1 = wp.tile([1, 128], MT, tag="ones1")
            if use_bf16:
                nc.gpsimd.memset(ones1[:].bitcast(mybir.dt.uint16), 0x3F80)
            else:
                nc.gpsimd.memset(ones1[:].bitcast(mybir.dt.uint32), 0x3F800000)
            gb_sb = wp.tile([1, 4 * H], MT, tag="gb")
            nc.gpsimd.dma_start(
                mm(gb_sb[:]).rearrange("q (l n) -> q l n", l=4), gbias[:, :]
            )

        def build_hh(hh_tile, src_tile, n_kc, pad):
            for kc in range(n_kc):
                src = src_tile[:, kc * (NN + 2 * pad) + pad
                               : kc * (NN + 2 * pad) + pad + NN]
                dst = hh_tile[:, kc * NN : (kc + 1) * NN]
                sv = src.rearrange("p (r c) -> p r c", c=GRID)
                dv = dst.rearrange("p (r c) -> p r c", c=GRID)
                nc.gpsimd.tensor_add(
                    mm(dv[:, :, 1:31]), sv[:, :, 0:30], sv[:, :, 2:32]
                )
                nc.gpsimd.tensor_copy(mm(dv[:, :, 0:1]), sv[:, :, 1:2])
                nc.gpsimd.tensor_copy(mm(dv[:, :, 31:32]), sv[:, :, 30:31])

        def emit_layer_mms(z, b, lhs_tile, hh_tile, n_kc, rhs_of_kc, l):
            mms = []
            for kc in range(n_kc):
                mms.append(
                    (hh_tile[:, kc * NN + b * 128 : kc * NN + b * 128 + 128],
                     rhs_of_kc(kc))
                )
            for kc in range(n_kc):
                base = kc * HW + b * 128
                mms.append((lhs_tile[:, base : base + 128], rhs_of_kc(kc)))
            for kc in range(n_kc):
                base = kc * HW + b * 128 + 64
                mms.append((lhs_tile[:, base : base + 128], rhs_of_kc(kc)))
            n = len(mms) + (1 if has_gin_bias else 0)
            for i, (lhsT, rhs) in enumerate(mms):
                nc.tensor.matmul(
                    z[:, :], mm(lhsT), mm(rhs),
                    start=(i == 0), stop=(i == n - 1),
                )
            if has_gin_bias:
                nc.tensor.matmul(
                    z[:, :], mm(ones1[0:1, 0:128]),
                    mm(gb_sb[0:1, l * H : (l + 1) * H]),
                    start=False, stop=True,
                )

        def prep_x(s):
            x_nm = px.tile([128, 256], MT, tag="xnm")
            dma = nc.gpsimd.dma_start if use_bf16 else nc.sync.dma_start
            dma(
                x_nm[:].rearrange("p (b f) -> p b f", f=F_IN),
                obs[s, NN:OBS_W].rearrange("(b p f) -> p b f", p=128, f=F_IN),
            )
            x_fm = px.tile([F_IN, HW], MT, tag="xfm")
            nc.gpsimd.memset(x_fm[:, 0:PAD].bitcast(GI), 0)
            nc.gpsimd.memset(x_fm[:, PAD + NN : HW].bitcast(GI), 0)
            for half in range(2):
                x_tfm = ptf.tile([F_IN, 512], MT, tag="tf")
                for i in range(4):
                    b = half * 4 + i
                    nc.tensor.transpose(
                        x_tfm[:, i * 128 : (i + 1) * 128],
                        x_nm[:, b * F_IN : (b + 1) * F_IN],
                        ident[:],
                    )
                nc.scalar.copy(
                    mm(x_fm[:, PAD + half * 512 : PAD + (half + 1) * 512]),
                    x_tfm[:],
                )
            hh_x = px.tile([F_IN, NN], MT, tag="hhx")
            build_hh(hh_x, x_fm, 1, PAD)
            return {"s": s, "x_fm": x_fm, "hh_x": hh_x, "h": []}

        def layer_mm_phase(st, l):
            if l == 0:
                n_kc = 1
                prev, prev_hh = st["x_fm"], st["hh_x"]
                rhs_of_kc = lambda kc: w0_sb[:, :]
            else:
                n_kc = 2
                prev, prev_hh = st["h"][l - 1], st["hh"]
                wl = wl_sb[l - 1]
                rhs_of_kc = lambda kc, wl=wl: wl[:, kc * H : (kc + 1) * H]

            t_nm = ph.tile([128, NB * H], MT, tag="tnm")
            for bp in range(4):
                zs = []
                for b in (2 * bp, 2 * bp + 1):
                    z = pz.tile([128, H], FP, tag="z")
                    emit_layer_mms(z, b, prev, prev_hh, n_kc, rhs_of_kc, l)
                    zs.append(z)
                mvp = pst.tile([128, 4], FP, tag="mv")
                for i, z in enumerate(zs):
                    st6 = pst.tile([128, 6], FP, tag="st6")
                    nc.vector.bn_stats(st6[:], z[:, :])
                    nc.vector.bn_aggr(mvp[:, 2 * i : 2 * i + 2], st6[:])
                sdp = pst.tile([128, 2], FP, tag="sd")
                var_view = mvp[:].rearrange("p (b t) -> p t b", t=2)[:, 1, :]
                nc.scalar.activation(
                    sdp[:], var_view, AF.Sqrt, bias=eps_sb[:, 0:1], scale=1.0
                )
                invp = pst.tile([128, 2], FP, tag="inv")
                nc.vector.reciprocal(invp[:], sdp[:])
                for i, z in enumerate(zs):
                    b = 2 * bp + i
                    nc.vector.tensor_scalar(
                        out=t_nm[:, b * H : (b + 1) * H],
                        in0=z[:, :],
                        scalar1=mvp[:, 2 * i : 2 * i + 1],
                        scalar2=invp[:, i : i + 1],
                        op0=OP.subtract,
                        op1=OP.mult,
                    )
            st["t_nm"] = t_nm

        def layer_tr_phase(st, l):
            t_nm = st.pop("t_nm")
            h_t = ph.tile([128, 2 * HW], MT, tag=f"h{l}")
            nc.gpsimd.memset(h_t[:, 0:PAD].bitcast(GI), 0)
            nc.gpsimd.memset(h_t[:, PAD + NN : HW + PAD].bitcast(GI), 0)
            nc.gpsimd.memset(h_t[:, HW + PAD + NN : 2 * HW].bitcast(GI), 0)
            for half in range(2):
                for c in range(2):
                    tf = ptf.tile([128, 512], MT, tag="tf", name="tfc")
                    for i in range(4):
                        b = half * 4 + i
                        nc.tensor.transpose(
                            tf[:, i * 128 : (i + 1) * 128],
                            t_nm[:, b * H + c * 128 : b * H + c * 128 + 128],
                            ident[:],
                        )
                    nc.scalar.activation(
                        mm(h_t[:, c * HW + PAD + half * 512
                               : c * HW + PAD + (half + 1) * 512]),
                        tf[:],
                        AF.Relu,
                        scale=gg_sb[:, l * 2 + c : l * 2 + c + 1],
                        bias=bb_sb[:, l * 2 + c : l * 2 + c + 1],
                    )
            st["h"].append(h_t)
            if l < 3:
                hh_t = ph.tile([128, 2 * NN], MT, tag="hh")
                build_hh(hh_t, h_t, 2, PAD)
                st["hh"] = hh_t

        def unit_w1(st):
            z_sb = ph.tile([128, 4096], MT, tag="zsb")
            for m in range(4):
                for c2 in range(2):
                    zw1 = ptf.tile([128, 512], FP, tag="tf")
                    for kc in range(9):
                        if kc == 0:
                            lhsT = w1x_sb[:, m * 128 : (m + 1) * 128]
                            rt, roff = st["x_fm"], 0
                        else:
                            j = kc - 1
                            lhsT = w1h_sb[:, j * 512 + m * 128
                                          : j * 512 + (m + 1) * 128]
                            rt, roff = st["h"][j // 2], (j % 2) * HW
                        nc.tensor.matmul(
                            zw1[:, :],
                            mm(lhsT),
                            mm(rt[:, roff + PAD + c2 * 512
                                   : roff + PAD + (c2 + 1) * 512]),
                            start=(kc == 0), stop=(kc == 8),
                        )
                    nc.scalar.activation(
                        mm(z_sb[:, m * NN + c2 * 512 : m * NN + (c2 + 1) * 512]),
                        zw1[:],
                        AF.Relu,
                        scale=bns_sb[:, m : m + 1],
                        bias=bnt_sb[:, m : m + 1],
                    )
            st["z_sb"] = z_sb

        def unit_w2(st):
            s = st["s"]
            z_sb = st["z_sb"]
            y_s = pfin.tile([1, NN], FP, tag="ys", bufs=2)
            for c2 in range(2):
                yp = pz.tile([1, 512], FP, tag="z")
                for m in range(4):
                    nc.tensor.matmul(
                        yp[0:1, :],
                        mm(w2_sb[:, m : m + 1]),
                        mm(z_sb[:, m * NN + c2 * 512 : m * NN + (c2 + 1) * 512]),
                        start=(m == 0), stop=(m == 3),
                    )
                nc.vector.tensor_copy(y_s[:, c2 * 512 : (c2 + 1) * 512], yp[0:1, :])
            if b2_val != 0.0:
                nc.scalar.add(y_s[:], y_s[:], b2_val)
            m_s = pfin.tile([1, NN], FP, tag="ms", bufs=2)
            nc.sync.dma_start(m_s[:], obs[s : s + 1, 0:NN])
            yf = pfin.tile([1, NN], FP, tag="yfin", bufs=2)
            nc.gpsimd.memset(yf[:], MIN_VAL)
            nc.vector.copy_predicated(yf[:], m_s[:].bitcast(mybir.dt.uint32), y_s[:])
            nc.sync.dma_start(y_out[s : s + 1, :], yf[:])

        for p in range(S // 2):
            sts = [prep_x(2 * p), prep_x(2 * p + 1)]
            for l in range(4):
                for st in sts:
                    layer_mm_phase(st, l)
                for st in sts:
                    layer_tr_phase(st, l)
            for st in sts:
                unit_w1(st)
            for st in sts:
                unit_w2(st)

    nc.finalize()
    return nc


_BUILD_CACHE = {}


def _get_nc(has_gin_bias: bool, b2_val: float, use_bf16: bool) -> bass.Bass:
    key = (has_gin_bias, float(b2_val), use_bf16)
    if key not in _BUILD_CACHE:
        _BUILD_CACHE[key] = _build(has_gin_bias, b2_val, use_bf16)
    return _BUILD_CACHE[key]


def prep_maps(observations, W0, b0, g0, be0, Ws, bs, gs, bes,
              W1, b1, bn_g, bn_b, bn_m, bn_v, W2, b2, **_ignored):
    obs = np.ascontiguousarray(np.asarray(observations, np.float32))
    W0 = np.ascontiguousarray(np.asarray(W0, np.float32))
    Ws = np.asarray(Ws, np.float32)
    W1 = np.asarray(W1, np.float32)
    W2 = np.asarray(W2, np.float32)
    gg = np.ascontiguousarray(np.stack(
        [np.asarray(g0, np.float32)] + [np.asarray(gs, np.float32)[i] for i in range(3)]))
    bb = np.ascontiguousarray(np.stack(
        [np.asarray(be0, np.float32)] + [np.asarray(bes, np.float32)[i] for i in range(3)]))
    gbias = np.ascontiguousarray(np.stack(
        [np.asarray(b0, np.float32)] + [np.asarray(bs, np.float32)[i] for i in range(3)]))
    has_gin_bias = bool(np.any(gbias != 0.0))
    bn_scale = (np.asarray(bn_g, np.float32)
                / np.sqrt(np.asarray(bn_v, np.float32) + EPS_BN)).astype(np.float32)
    bn_shift = ((np.asarray(b1, np.float32) - np.asarray(bn_m, np.float32)) * bn_scale
                + np.asarray(bn_b, np.float32)).astype(np.float32)
    b2_val = float(np.asarray(b2, np.float32).reshape(-1)[0])

    ws_r = np.ascontiguousarray(Ws.reshape(3, 2, 128, H))
    w1x = np.ascontiguousarray(W1[:F_IN])
    w1h = np.ascontiguousarray(W1[F_IN:].reshape(8, 128, 512))
    w2r = np.ascontiguousarray(W2.reshape(4, 128))

    shared = {
        "w0": W0, "ws": ws_r, "w1x": w1x, "w1h": w1h, "w2": w2r,
        "gg": gg, "bb": bb, "bns": bn_scale, "bnt": bn_shift,
    }
    if has_gin_bias:
        shared["gbias"] = gbias
    in_maps = []
    for c in range(NCORE):
        m = dict(shared)
        m["obs"] = np.ascontiguousarray(obs[c * S : (c + 1) * S])
        in_maps.append(m)
    return in_maps, has_gin_bias, b2_val


def kernel(**inputs) -> np.ndarray:
    global LAST_EXEC_NS
    in_maps, has_gin_bias, b2_val = prep_maps(**inputs)
    nc = _get_nc(has_gin_bias, b2_val, USE_BF16)
    res = run_bass_kernel_spmd(
        nc, in_maps, list(range(NCORE)), trace=PROFILE, **TRACE_KWARGS
    )
    LAST_EXEC_NS = res.exec_time_ns
    y = np.concatenate([res.results[c]["y"] for c in range(NCORE)], axis=0)
    return y.reshape(B, NN).astype(np.float32)



# revision 13
# speedup vs baseline: 1.0178x; 1.0178x over previous
import numpy as np

import concourse.bass as bass
import concourse.bacc as bacc
import concourse.mybir as mybir
import concourse.tile as tile
from concourse.bass_utils import run_bass_kernel_spmd
from concourse.masks import make_identity

FP = mybir.dt.float32
BF = mybir.dt.bfloat16
U16 = mybir.dt.uint16
AF = mybir.ActivationFunctionType
OP = mybir.AluOpType

GRID = 32
NN = 1024
F_IN = 32
H = 256
B = 64
S = 8
NCORE = 8
OBS_W = NN + NN * F_IN
MIN_VAL = -10000000.0
EPS_LN = 1e-5
EPS_BN = 1e-5
PAD = 32
HW = NN + 2 * PAD

USE_BF16 = True
PROFILE = False
LAST_EXEC_NS = None
TRACE_KWARGS = {}


def _build(cfg, unused_b2=None, unused_bf16=None) -> bass.Bass:
    has_gin_bias = cfg["has_gin_bias"]
    fast_ln = cfg["fast_ln"]
    gscal = cfg["gscal"]
    b2_val = cfg["b2_val"]

    nc = bacc.Bacc("TRN2", target_bir_lowering=False, debug=False)

    obs = nc.declare_dram_parameter("obs", [S, OBS_W], FP, isOutput=False)
    w0 = nc.declare_dram_parameter("w0", [F_IN, H], FP, isOutput=False)
    ws = nc.declare_dram_parameter("ws", [3, H, H], FP, isOutput=False)
    w1x = nc.declare_dram_parameter("w1x", [F_IN, 512], FP, isOutput=False)
    w1h = nc.declare_dram_parameter("w1h", [8, 128, 512], FP, isOutput=False)
    w2 = nc.declare_dram_parameter("w2", [4, 128], FP, isOutput=False)
    bns = nc.declare_dram_parameter("bns", [512], FP, isOutput=False)
    bnt = nc.declare_dram_parameter("bnt", [512], FP, isOutput=False)
    if has_gin_bias:
        gbias = nc.declare_dram_parameter("gbias", [4, H], FP, isOutput=False)
    if not fast_ln:
        grows = nc.declare_dram_parameter("grows", [4, H], FP, isOutput=False)
        bbr = nc.declare_dram_parameter("bbr", [4, H], FP, isOutput=False)
    y_out = nc.declare_dram_parameter("y", [S, NN], FP, isOutput=True)

    from contextlib import ExitStack

    with tile.TileContext(nc) as tc, ExitStack() as ctx:
        wp = ctx.enter_context(tc.tile_pool(name="w", bufs=1))
        px = ctx.enter_context(tc.tile_pool(name="px", bufs=2))
        pxf = ctx.enter_context(tc.tile_pool(name="pxf", bufs=5))
        pagx = ctx.enter_context(tc.tile_pool(name="pagx", bufs=2))
        ph = ctx.enter_context(tc.tile_pool(name="ph", bufs=5))
        phh = ctx.enter_context(tc.tile_pool(name="phh", bufs=2))
        pagg = ctx.enter_context(tc.tile_pool(name="pagg", bufs=3))
        pzsq = ctx.enter_context(tc.tile_pool(name="pzsq", bufs=3))
        psr = ctx.enter_context(tc.tile_pool(name="psr", bufs=4))
        psgs = ctx.enter_context(tc.tile_pool(name="psgs", bufs=3))
        pzsb = ctx.enter_context(tc.tile_pool(name="pzsb", bufs=2))
        pfin = ctx.enter_context(tc.tile_pool(name="pfin", bufs=2))
        pz = ctx.enter_context(tc.tile_pool(name="pz", bufs=4, space="PSUM"))
        pvar = ctx.enter_context(tc.tile_pool(name="pvar", bufs=2, space="PSUM"))
        psg = ctx.enter_context(tc.tile_pool(name="psg", bufs=2, space="PSUM"))

        ident = wp.tile([128, 128], FP, tag="id")
        make_identity(nc, ident[:])

        ones_row = wp.tile([1, 512], BF, tag="ones_row")
        nc.gpsimd.memset(ones_row[:].bitcast(U16), 0x3F80)
        ones_col = wp.tile([128, 1], BF, tag="ones_col")
        nc.gpsimd.memset(ones_col[:].bitcast(U16), 0x3F80)

        zero_col = wp.tile([128, 1], FP, tag="zero_col")
        nc.gpsimd.memset(zero_col[:], 0.0)
        zero_row = wp.tile([1, 1], FP, tag="zero_row")
        nc.gpsimd.memset(zero_row[:], 0.0)
        eps_row = wp.tile([1, 4], FP, tag="eps_row")
        for l in range(4):
            gl = gscal[l] if fast_ln else 1.0
            nc.gpsimd.memset(eps_row[:, l: l + 1], EPS_LN / (gl * gl))

        w0_sb = wp.tile([F_IN, H], BF, tag="w0")
        nc.gpsimd.dma_start(w0_sb[:], w0[:, :])

        wl_sb = wp.tile([128, 3 * 512], BF, tag="wl")
        for l in range(3):
            nc.gpsimd.dma_start(
                wl_sb[:, l * 512:(l + 1) * 512]
                .rearrange("p (ci co c) -> p ci co c", ci=2, co=2),
                ws[l].rearrange("(ci p) (co c) -> p ci co c", p=128, c=128),
            )

        w1x_sb = wp.tile([F_IN, 512], BF, tag="w1x")
        nc.gpsimd.dma_start(w1x_sb[:], w1x[:, :])
        w1h_sb = wp.tile([128, 8 * 512], BF, tag="w1h")
        nc.gpsimd.dma_start(
            w1h_sb[:].rearrange("p (j m) -> p j m", j=8),
            w1h[:, :, :].rearrange("j p m -> p j m"),
        )
        w2_sb = wp.tile([128, 4], BF, tag="w2")
        nc.gpsimd.dma_start(w2_sb[:], w2[:, :].rearrange("k p -> p k"))

        bns_sb = wp.tile([128, 4], FP, tag="bns")
        nc.sync.dma_start(bns_sb[:], bns[:].rearrange("(m p) -> p m", p=128))
        bnt_sb = wp.tile([128, 4], FP, tag="bnt")
        nc.sync.dma_start(bnt_sb[:], bnt[:].rearrange("(m p) -> p m", p=128))

        if has_gin_bias:
            gb_row = wp.tile([1, 4 * H], BF, tag="gb")
            nc.gpsimd.dma_start(
                gb_row[:].rearrange("q (l n) -> q l n", l=4), gbias[:, :]
            )
        if not fast_ln:
            g_row = wp.tile([1, 4 * H], BF, tag="grow")
            nc.gpsimd.dma_start(
                g_row[:].rearrange("q (l n) -> q l n", l=4), grows[:, :]
            )
            bb_sb = wp.tile([128, 8], FP, tag="bb")
            nc.sync.dma_start(
                bb_sb[:].rearrange("p (l c) -> p l c", c=2),
                bbr[:, :].rearrange("l (c p) -> p l c", p=128),
            )

        def wchunk(l, ci, co):
            if l == 0:
                return w0_sb[:, co * 128: co * 128 + 128]
            return wl_sb[:, (l - 1) * 512 + ci * 256 + co * 128
                         : (l - 1) * 512 + ci * 256 + co * 128 + 128]

        def build_hh(hh_ap, src_ap):
            sv = src_ap.rearrange("p (r c) -> p r c", c=GRID)
            dv = hh_ap.rearrange("p (r c) -> p r c", c=GRID)
            nc.gpsimd.tensor_add(dv[:, :, 1:31], sv[:, :, 0:30], sv[:, :, 2:32])
            nc.gpsimd.tensor_copy(dv[:, :, 0:1], sv[:, :, 1:2])
            nc.gpsimd.tensor_copy(dv[:, :, 31:32], sv[:, :, 30:31])

        def prep_x(s):
            x_nm = px.tile([128, 256], FP, tag="xnm")
            nc.sync.dma_start(
                x_nm[:].rearrange("p (b f) -> p b f", f=F_IN),
                obs[s, NN:OBS_W].rearrange("(b p f) -> p b f", p=128, f=F_IN),
            )
            x_fm = pxf.tile([F_IN, HW], BF, tag="xfm")
            nc.gpsimd.memset(x_fm[:, 0:PAD].bitcast(U16), 0)
            nc.gpsimd.memset(x_fm[:, PAD + NN: HW].bitcast(U16), 0)
            for half in range(2):
                x_tfm = pz.tile([F_IN, 512], FP, tag="z", name="xtf")
                for i in range(4):
                    b = half * 4 + i
                    nc.tensor.transpose(
                        x_tfm[:, i * 128:(i + 1) * 128],
                        x_nm[:, b * F_IN:(b + 1) * F_IN],
                        ident[:],
                    )
                nc.scalar.copy(
                    x_fm[:, PAD + half * 512: PAD + (half + 1) * 512], x_tfm[:]
                )
            hh_x = pagx.tile([F_IN, NN], BF, tag="hhx")
            build_hh(hh_x[:, :], x_fm[:, PAD: PAD + NN])
            agg_x = pagx.tile([F_IN, NN], BF, tag="aggx")
            nc.vector.tensor_add(
                agg_x[:, :], x_fm[:, 0: NN], x_fm[:, 2 * PAD: 2 * PAD + NN]
            )
            nc.vector.tensor_add(agg_x[:, :], agg_x[:, :], hh_x[:, :])
            return {"s": s, "x_fm": x_fm, "agg_x": agg_x, "h": []}

        def layer_begin(st, l):
            st["zsq"] = pzsq.tile([128, 2048], BF, tag="zsq", name="zsq")
            h_t = ph.tile([128, 2 * HW], BF, tag=f"h{l}", name="ht")
            nc.gpsimd.memset(h_t[:, 0:PAD].bitcast(U16), 0)
            nc.gpsimd.memset(h_t[:, PAD + NN: HW + PAD].bitcast(U16), 0)
            nc.gpsimd.memset(h_t[:, HW + PAD + NN: 2 * HW].bitcast(U16), 0)
            st["h"].append(h_t)

        def layer_mm(st, l, half):
            kc = 1 if l == 0 else 2
            toff = half * 512
            zz = []
            for co in range(2):
                z = pz.tile([128, 512], FP, tag="z")
                for ci in range(kc):
                    if l == 0:
                        rhs = st["agg_x"][:, toff: toff + 512]
                    else:
                        rhs = st["agg"][:, ci * NN + toff: ci * NN + toff + 512]
                    nc.tensor.matmul(
                        z[:, :], wchunk(l, ci, co), rhs,
                        start=(ci == 0),
                        stop=(ci == kc - 1 and not has_gin_bias),
                    )
                if has_gin_bias:
                    nc.tensor.matmul(
                        z[:, :],
                        gb_row[0:1, l * H + co * 128: l * H + co * 128 + 128],
                        ones_row[0:1, 0:512],
                        start=False, stop=True,
                    )
                zz.append(z)
            for co in range(2):
                nc.scalar.activation(
                    st["zsq"][:, co * NN + toff: co * NN + toff + 512],
                    zz[co][:, :], AF.Square, bias=zero_col[:, 0:1],
                )
            st["zz%d" % half] = zz

        def layer_ln(st, l, half):
            toff = half * 512
            zz = st.pop("zz%d" % half)
            h_t = st["h"][l]
            var = pvar.tile([1, 512], FP, tag="var")
            for co in range(2):
                nc.tensor.matmul(
                    var[0:1, :], ones_col[:, 0:1],
                    st["zsq"][:, co * NN + toff: co * NN + toff + 512],
                    start=(co == 0), stop=(co == 1),
                )
            s_row = psr.tile([1, 512], BF, tag="srow")
            lnv = psr.tile([1, 512], FP, tag="srow", name="lnv")
            if fast_ln:
                g = gscal[l]
                nc.scalar.activation(
                    lnv[:], var[0:1, :], AF.Ln,
                    bias=eps_row[0:1, l: l + 1], scale=1.0 / (H * g * g),
                )
                nc.scalar.activation(
                    s_row[:], lnv[:], AF.Exp, bias=zero_row[0:1, 0:1], scale=-0.5
                )
                sg = psg.tile([128, 512], FP, tag="sg")
                nc.tensor.matmul(
                    sg[:, :], ones_row[0:1, 0:128], s_row[0:1, :],
                    start=True, stop=True,
                )
                sgs = psgs.tile([128, 512], BF, tag="sgs")
                nc.scalar.copy(sgs[:], sg[:, :])
                for co in range(2):
                    nc.vector.scalar_tensor_tensor(
                        h_t[:, co * HW + PAD + toff: co * HW + PAD + toff + 512],
                        zz[co][:, :], 0.0, sgs[:],
                        op0=OP.max, op1=OP.mult,
                    )
            else:
                nc.scalar.activation(
                    lnv[:], var[0:1, :], AF.Ln,
                    bias=eps_row[0:1, l: l + 1], scale=1.0 / H,
                )
                nc.scalar.activation(
                    s_row[:], lnv[:], AF.Exp, bias=zero_row[0:1, 0:1], scale=-0.5
                )
                for co in range(2):
                    sg = psg.tile([128, 512], FP, tag="sg")
                    nc.tensor.matmul(
                        sg[:, :],
                        g_row[0:1, l * H + co * 128: l * H + co * 128 + 128],
                        s_row[0:1, :], start=True, stop=True,
                    )
                    sgs = psgs.tile([128, 512], BF, tag="sgs")
                    nc.scalar.copy(sgs[:], sg[:, :])
                    tmb = psgs.tile([128, 512], BF, tag="sgs", name="tmb")
                    nc.vector.tensor_tensor(
                        tmb[:], zz[co][:, :], sgs[:], op=OP.mult
                    )
                    nc.vector.tensor_scalar(
                        out=h_t[:, co * HW + PAD + toff
                                : co * HW + PAD + toff + 512],
                        in0=tmb[:],
                        scalar1=bb_sb[:, l * 2 + co: l * 2 + co + 1],
                        scalar2=0.0,
                        op0=OP.add, op1=OP.max,
                    )
            if half == 1 and l < 3:
                hh_t = phh.tile([128, 2 * NN], BF, tag="hh")
                agg_t = pagg.tile([128, 2 * NN], BF, tag="agg")
                for co in range(2):
                    build_hh(
                        hh_t[:, co * NN:(co + 1) * NN],
                        h_t[:, co * HW + PAD: co * HW + PAD + NN],
                    )
                    nc.vector.tensor_add(
                        agg_t[:, co * NN:(co + 1) * NN],
                        h_t[:, co * HW: co * HW + NN],
                        h_t[:, co * HW + 2 * PAD: co * HW + 2 * PAD + NN],
                    )
                    nc.vector.tensor_add(
                        agg_t[:, co * NN:(co + 1) * NN],
                        agg_t[:, co * NN:(co + 1) * NN],
                        hh_t[:, co * NN:(co + 1) * NN],
                    )
                st["agg"] = agg_t

        def layer_round(group, l):
            for st in group:
                layer_begin(st, l)
            q = []
            for st in group:
                for half in range(2):
                    q.append((st, half))
            done_a = 0
            done_b = 0
            n = len(q)
            order = []
            while done_a < n or done_b < n:
                if done_a < n and done_a - done_b < 2:
                    order.append(("A", q[done_a])); done_a += 1
                else:
                    order.append(("B", q[done_b])); done_b += 1
            for ph_, (st, half) in order:
                if ph_ == "A":
                    layer_mm(st, l, half)
                else:
                    layer_ln(st, l, half)

        def w1_chunk(st, m):
            if m == 0:
                st["zsb"] = pzsb.tile([128, 4096], BF, tag="zsb", name="zsb")
            zsb = st["zsb"]
            zw = [pz.tile([128, 512], FP, tag="z", name="zw") for _ in range(2)]
            for kc9 in range(9):
                if kc9 == 0:
                    lhsT = w1x_sb[:, m * 128:(m + 1) * 128]
                else:
                    j = kc9 - 1
                    lhsT = w1h_sb[:, j * 512 + m * 128: j * 512 + m * 128 + 128]
                for half in range(2):
                    toff = half * 512
                    if kc9 == 0:
                        rhs = st["x_fm"][:, PAD + toff: PAD + toff + 512]
                    else:
                        j = kc9 - 1
                        co = j % 2
                        rhs = st["h"][j // 2][:, co * HW + PAD + toff
                                              : co * HW + PAD + toff + 512]
                    nc.tensor.matmul(
                        zw[half][:, :], lhsT, rhs,
                        start=(kc9 == 0), stop=(kc9 == 8),
                    )
            for half in range(2):
                nc.scalar.activation(
                    zsb[:, m * NN + half * 512: m * NN + half * 512 + 512],
                    zw[half][:, :], AF.Relu,
                    scale=bns_sb[:, m: m + 1], bias=bnt_sb[:, m: m + 1],
                )

        def head_finish(st):
            s = st["s"]
            zsb = st["zsb"]
            y_s = pfin.tile([1, NN], FP, tag="ys")
            for c2 in range(2):
                yp = pvar.tile([1, 512], FP, tag="var", name="yp")
                for m in range(4):
                    nc.tensor.matmul(
                        yp[0:1, :], w2_sb[:, m: m + 1],
                        zsb[:, m * NN + c2 * 512: m * NN + c2 * 512 + 512],
                        start=(m == 0), stop=(m == 3),
                    )
                nc.vector.tensor_copy(y_s[:, c2 * 512:(c2 + 1) * 512], yp[0:1, :])
            if b2_val != 0.0:
                nc.scalar.add(y_s[:], y_s[:], b2_val)
            m_s = pfin.tile([1, NN], FP, tag="ms")
            nc.sync.dma_start(m_s[:], obs[s: s + 1, 0:NN])
            yf = pfin.tile([1, NN], FP, tag="yfin")
            nc.gpsimd.memset(yf[:], MIN_VAL)
            nc.vector.copy_predicated(yf[:], m_s[:].bitcast(mybir.dt.uint32), y_s[:])
            nc.sync.dma_start(y_out[s: s + 1, :], yf[:])

        def head_units(st):
            return [lambda m=m, st=st: w1_chunk(st, m) for m in range(4)] \
                + [lambda st=st: head_finish(st)]

        headq = []

        def flush(k=None):
            n = len(headq) if k is None else min(k, len(headq))
            for _ in range(n):
                headq.pop(0)()

        prev = [prep_x(0), prep_x(1)]
        for l in range(4):
            layer_round(prev, l)
        for p in range(1, 4):
            for st in prev:
                headq.extend(head_units(st))
            cur = [prep_x(2 * p), prep_x(2 * p + 1)]
            for l in range(4):
                layer_round(cur, l)
                flush(3)
            flush()
            prev = cur
        for st in prev:
            headq.extend(head_units(st))
        flush()

    nc.finalize()
    return nc


_BUILD_CACHE = {}
_CFG = None


def _get_nc(has_gin_bias=None, b2_val=None, use_bf16=None) -> bass.Bass:
    cfg = _CFG
    key = (cfg["has_gin_bias"], cfg["fast_ln"], cfg["gscal"], cfg["b2_val"])
    if key not in _BUILD_CACHE:
        _BUILD_CACHE[key] = _build(cfg)
    return _BUILD_CACHE[key]


def prep_maps(observations, W0, b0, g0, be0, Ws, bs, gs, bes,
              W1, b1, bn_g, bn_b, bn_m, bn_v, W2, b2, **_ignored):
    global _CFG
    obs = np.ascontiguousarray(np.asarray(observations, np.float32))
    W0 = np.asarray(W0, np.float32)
    Ws = np.asarray(Ws, np.float32)
    W1 = np.asarray(W1, np.float32)
    W2 = np.asarray(W2, np.float32)

    W0c = np.ascontiguousarray(W0 - W0.mean(axis=1, keepdims=True))
    Wsc = np.ascontiguousarray(Ws - Ws.mean(axis=2, keepdims=True))

    gg = np.stack([np.asarray(g0, np.float32)]
                  + [np.asarray(gs, np.float32)[i] for i in range(3)])
    bb = np.stack([np.asarray(be0, np.float32)]
                  + [np.asarray(bes, np.float32)[i] for i in range(3)])
    graw = np.stack([np.asarray(b0, np.float32)]
                    + [np.asarray(bs, np.float32)[i] for i in range(3)])
    gbias = np.ascontiguousarray(graw - graw.mean(axis=1, keepdims=True))
    has_gin_bias = bool(np.any(np.abs(gbias) > 0.0))

    g_const = all(np.all(gg[i] == gg[i][0]) for i in range(4))
    fast_ln = bool(np.all(bb == 0.0) and g_const
                   and all(gg[i][0] > 0 for i in range(4)))
    gscal = tuple(float(gg[i][0]) for i in range(4)) if fast_ln else (0.0,) * 4

    bn_scale = (np.asarray(bn_g, np.float32)
                / np.sqrt(np.asarray(bn_v, np.float32) + EPS_BN)).astype(np.float32)
    bn_shift = ((np.asarray(b1, np.float32) - np.asarray(bn_m, np.float32)) * bn_scale
                + np.asarray(bn_b, np.float32)).astype(np.float32)
    b2_val = float(np.asarray(b2, np.float32).reshape(-1)[0])

    w1x = np.ascontiguousarray(W1[:F_IN])
    w1h = np.ascontiguousarray(W1[F_IN:].reshape(8, 128, 512))
    w2r = np.ascontiguousarray(W2.reshape(4, 128))

    _CFG = {"has_gin_bias": has_gin_bias, "fast_ln": fast_ln,
            "gscal": gscal, "b2_val": b2_val}

    shared = {
        "w0": W0c, "ws": Wsc, "w1x": w1x, "w1h": w1h, "w2": w2r,
        "bns": bn_scale, "bnt": bn_shift,
    }
    if has_gin_bias:
        shared["gbias"] = gbias
    if not fast_ln:
        shared["grows"] = np.ascontiguousarray(gg)
        shared["bbr"] = np.ascontiguousarray(bb)
    in_maps = []
    for c in range(NCORE):
        m = dict(shared)
        m["obs"] = np.ascontiguousarray(obs[c * S: (c + 1) * S])
        in_maps.append(m)
    return in_maps, has_gin_bias, b2_val


def kernel(**inputs) -> np.ndarray:
    global LAST_EXEC_NS
    in_maps, has_gin_bias, b2_val = prep_maps(**inputs)
    nc = _get_nc(has_gin_bias, b2_val, USE_BF16)
    res = run_bass_kernel_spmd(
        nc, in_maps, list(range(NCORE)), trace=PROFILE, **TRACE_KWARGS
    )
    LAST_EXEC_NS = res.exec_time_ns
    y = np.concatenate([res.results[c]["y"] for c in range(NCORE)], axis=0)
    return y.reshape(B, NN).astype(np.float32)


# revision 17
# speedup vs baseline: 1.1935x; 1.1727x over previous
import numpy as np

import concourse.bass as bass
import concourse.bacc as bacc
import concourse.mybir as mybir
import concourse.tile as tile
from concourse.bass_utils import run_bass_kernel_spmd
from concourse.masks import make_identity

FP = mybir.dt.float32
BF = mybir.dt.bfloat16
U16 = mybir.dt.uint16
AF = mybir.ActivationFunctionType
OP = mybir.AluOpType

GRID = 32
NN = 1024
F_IN = 32
H = 256
B = 64
S = 8
NCORE = 8
OBS_W = NN + NN * F_IN
MIN_VAL = -10000000.0
EPS_LN = 1e-5
EPS_BN = 1e-5
PAD = 32
HW = NN + 2 * PAD

USE_BF16 = True
PROFILE = False
LAST_EXEC_NS = None
TRACE_KWARGS = {}

_ACT_PATCHED = False


def _patch_act_tables():
    global _ACT_PATCHED
    if _ACT_PATCHED:
        return
    import concourse.hw_specs as hw_specs

    orig = hw_specs.get_activation_tables

    def patched(arch):
        t = orig(arch)
        if "natural_log_exp_and_others" in t:
            both = t["natural_log_exp_and_others"]
            if AF.Ln in both and AF.Exp in both:
                for name, fns in t.items():
                    if name != "natural_log_exp_and_others":
                        fns.discard(AF.Exp)
                        fns.discard(AF.Ln)
        return t

    hw_specs.get_activation_tables = patched
    for mod in (bacc, bass):
        if getattr(mod, "get_activation_tables", None) is orig:
            mod.get_activation_tables = patched
    _ACT_PATCHED = True


def _build(cfg, unused_b2=None, unused_bf16=None) -> bass.Bass:
    has_gin_bias = cfg["has_gin_bias"]
    fast_ln = cfg["fast_ln"]
    gscal = cfg["gscal"]
    b2_val = cfg["b2_val"]

    _patch_act_tables()
    nc = bacc.Bacc("TRN2", target_bir_lowering=False, debug=False)

    obs = nc.declare_dram_parameter("obs", [S, OBS_W], FP, isOutput=False)
    w0 = nc.declare_dram_parameter("w0", [F_IN, H], FP, isOutput=False)
    ws = nc.declare_dram_parameter("ws", [3, H, H], FP, isOutput=False)
    w1x = nc.declare_dram_parameter("w1x", [F_IN, 512], FP, isOutput=False)
    w1h = nc.declare_dram_parameter("w1h", [8, 128, 512], FP, isOutput=False)
    w2 = nc.declare_dram_parameter("w2", [4, 128], FP, isOutput=False)
    bns = nc.declare_dram_parameter("bns", [512], FP, isOutput=False)
    bnt = nc.declare_dram_parameter("bnt", [512], FP, isOutput=False)
    if has_gin_bias:
        gbias = nc.declare_dram_parameter("gbias", [4, H], FP, isOutput=False)
    if not fast_ln:
        grows = nc.declare_dram_parameter("grows", [4, H], FP, isOutput=False)
        bbr = nc.declare_dram_parameter("bbr", [4, H], FP, isOutput=False)
    y_out = nc.declare_dram_parameter("y", [S, NN], FP, isOutput=True)

    from contextlib import ExitStack

    with tile.TileContext(nc) as tc, ExitStack() as ctx:
        wp = ctx.enter_context(tc.tile_pool(name="w", bufs=1))
        px = ctx.enter_context(tc.tile_pool(name="px", bufs=2))
        pxf = ctx.enter_context(tc.tile_pool(name="pxf", bufs=5))
        pagx = ctx.enter_context(tc.tile_pool(name="pagx", bufs=3))
        ph = ctx.enter_context(tc.tile_pool(name="ph", bufs=17))
        phh = ctx.enter_context(tc.tile_pool(name="phh", bufs=3))
        pzsq = ctx.enter_context(tc.tile_pool(name="pzsq", bufs=3))
        plnv = ctx.enter_context(tc.tile_pool(name="plnv", bufs=3))
        psgs = ctx.enter_context(tc.tile_pool(name="psgs", bufs=3))
        pzsb = ctx.enter_context(tc.tile_pool(name="pzsb", bufs=2))
        pfin = ctx.enter_context(tc.tile_pool(name="pfin", bufs=2))
        pz = ctx.enter_context(tc.tile_pool(name="pz", bufs=6, space="PSUM"))
        pvar = ctx.enter_context(tc.tile_pool(name="pvar", bufs=2, space="PSUM"))

        ident = wp.tile([128, 128], FP, tag="id")
        make_identity(nc, ident[:])

        ones_mat = wp.tile([128, 128], BF, tag="ones_mat")
        nc.gpsimd.memset(ones_mat[:].bitcast(U16), 0x3F80)
        ones_row = wp.tile([1, 512], BF, tag="ones_row")
        nc.gpsimd.memset(ones_row[:].bitcast(U16), 0x3F80)

        zero_col = wp.tile([128, 1], FP, tag="zero_col")
        nc.gpsimd.memset(zero_col[:], 0.0)
        eps_col = wp.tile([128, 4], FP, tag="eps_col")
        for l in range(4):
            gl = gscal[l] if fast_ln else 1.0
            nc.gpsimd.memset(eps_col[:, l: l + 1], EPS_LN / (gl * gl))

        w0_sb = wp.tile([F_IN, H], BF, tag="w0")
        nc.gpsimd.dma_start(w0_sb[:], w0[:, :])

        wl_sb = wp.tile([128, 3 * 512], BF, tag="wl")
        for l in range(3):
            nc.gpsimd.dma_start(
                wl_sb[:, l * 512:(l + 1) * 512]
                .rearrange("p (ci co c) -> p ci co c", ci=2, co=2),
                ws[l].rearrange("(ci p) (co c) -> p ci co c", p=128, c=128),
            )

        w1x_sb = wp.tile([F_IN, 512], BF, tag="w1x")
        nc.gpsimd.dma_start(w1x_sb[:], w1x[:, :])
        w1h_sb = wp.tile([128, 8 * 512], BF, tag="w1h")
        nc.gpsimd.dma_start(
            w1h_sb[:].rearrange("p (j m) -> p j m", j=8),
            w1h[:, :, :].rearrange("j p m -> p j m"),
        )
        w2_sb = wp.tile([128, 4], BF, tag="w2")
        nc.gpsimd.dma_start(w2_sb[:], w2[:, :].rearrange("k p -> p k"))

        bns_sb = wp.tile([128, 4], FP, tag="bns")
        nc.sync.dma_start(bns_sb[:], bns[:].rearrange("(m p) -> p m", p=128))
        bnt_sb = wp.tile([128, 4], FP, tag="bnt")
        nc.sync.dma_start(bnt_sb[:], bnt[:].rearrange("(m p) -> p m", p=128))

        if has_gin_bias:
            gb_row = wp.tile([1, 4 * H], BF, tag="gb")
            nc.gpsimd.dma_start(
                gb_row[:].rearrange("q (l n) -> q l n", l=4), gbias[:, :]
            )
        if not fast_ln:
            g_col = wp.tile([128, 8], FP, tag="gcol")
            nc.sync.dma_start(
                g_col[:].rearrange("p (l c) -> p l c", c=2),
                grows[:, :].rearrange("l (c p) -> p l c", p=128),
            )
            bb_sb = wp.tile([128, 8], FP, tag="bb")
            nc.sync.dma_start(
                bb_sb[:].rearrange("p (l c) -> p l c", c=2),
                bbr[:, :].rearrange("l (c p) -> p l c", p=128),
            )

        def wchunk(l, ci, co):
            if l == 0:
                return w0_sb[:, co * 128: co * 128 + 128]
            return wl_sb[:, (l - 1) * 512 + ci * 256 + co * 128
                         : (l - 1) * 512 + ci * 256 + co * 128 + 128]

        def build_hh(hh_ap, src_ap):
            sv = src_ap.rearrange("p (r c) -> p r c", c=GRID)
            dv = hh_ap.rearrange("p (r c) -> p r c", c=GRID)
            nc.gpsimd.tensor_add(dv[:, :, 1:31], sv[:, :, 0:30], sv[:, :, 2:32])
            nc.vector.tensor_copy(dv[:, :, 0:1], sv[:, :, 1:2])
            nc.vector.tensor_copy(dv[:, :, 31:32], sv[:, :, 30:31])

        def prep_x(s):
            x_nm = px.tile([128, 256], FP, tag="xnm")
            nc.sync.dma_start(
                x_nm[:].rearrange("p (b f) -> p b f", f=F_IN),
                obs[s, NN:OBS_W].rearrange("(b p f) -> p b f", p=128, f=F_IN),
            )
            x_fm = pxf.tile([F_IN, HW], BF, tag="xfm")
            nc.gpsimd.memset(x_fm[:, 0:PAD].bitcast(U16), 0)
            nc.gpsimd.memset(x_fm[:, PAD + NN: HW].bitcast(U16), 0)
            for half in range(2):
                x_tfm = pz.tile([F_IN, 512], FP, tag="z", name="xtf")
                for i in range(4):
                    b = half * 4 + i
                    nc.tensor.transpose(
                        x_tfm[:, i * 128:(i + 1) * 128],
                        x_nm[:, b * F_IN:(b + 1) * F_IN],
                        ident[:],
                    )
                nc.vector.tensor_copy(
                    x_fm[:, PAD + half * 512: PAD + (half + 1) * 512], x_tfm[:]
                )
            hh_x = pagx.tile([F_IN, NN], BF, tag="hhx")
            build_hh(hh_x[:, :], x_fm[:, PAD: PAD + NN])
            return {"s": s, "x_fm": x_fm, "hh_x": hh_x, "h": []}

        def layer_mm(st, l):
            kc = 1 if l == 0 else 2
            st["zsq"] = pzsq.tile([128, 2048], BF, tag="zsq", name="zsq")
            h_t = ph.tile([128, 2 * HW], BF, tag="ht", name="ht")
            nc.gpsimd.memset(h_t[:, 0:PAD].bitcast(U16), 0)
            nc.gpsimd.memset(h_t[:, PAD + NN: HW + PAD].bitcast(U16), 0)
            nc.gpsimd.memset(h_t[:, HW + PAD + NN: 2 * HW].bitcast(U16), 0)
            st["h"].append(h_t)
            zz = [[None, None], [None, None]]
            for half in range(2):
                toff = half * 512
                for co in range(2):
                    z = pz.tile([128, 512], FP, tag="z")
                    nmm = 3 * kc + (1 if has_gin_bias else 0)
                    i = 0
                    for ci in range(kc):
                        if l == 0:
                            hh_w = st["hh_x"][:, toff: toff + 512]
                            up_w = st["x_fm"][:, toff: toff + 512]
                            dn_w = st["x_fm"][:, 2 * PAD + toff: 2 * PAD + toff + 512]
                        else:
                            hprev = st["h"][l - 1]
                            hhprev = st["hh"]
                            hh_w = hhprev[:, ci * NN + toff: ci * NN + toff + 512]
                            up_w = hprev[:, ci * HW + toff: ci * HW + toff + 512]
                            dn_w = hprev[:, ci * HW + 2 * PAD + toff
                                         : ci * HW + 2 * PAD + toff + 512]
                        for rhs in (hh_w, up_w, dn_w):
                            nc.tensor.matmul(
                                z[:, :], wchunk(l, ci, co), rhs,
                                start=(i == 0), stop=(i == nmm - 1),
                            )
                            i += 1
                    if has_gin_bias:
                        nc.tensor.matmul(
                            z[:, :],
                            gb_row[0:1, l * H + co * 128: l * H + co * 128 + 128],
                            ones_row[0:1, 0:512],
                            start=False, stop=True,
                        )
                    zz[half][co] = z
                for co in range(2):
                    nc.scalar.activation(
                        st["zsq"][:, co * NN + toff: co * NN + toff + 512],
                        zz[half][co][:, :], AF.Square, bias=zero_col[:, 0:1],
                    )
            st["zz"] = zz

        def layer_ln_half(st, l, half):
            toff = half * 512
            zz = st["zz"]
            zsq = st["zsq"]
            h_t = st["h"][l]
            var = pvar.tile([128, 512], FP, tag="var")
            for co in range(2):
                nc.tensor.matmul(
                    var[:, :], ones_mat[:, :],
                    zsq[:, co * NN + toff: co * NN + toff + 512],
                    start=(co == 0), stop=(co == 1),
                )
            lnv = plnv.tile([128, 512], FP, tag="lnv")
            g = gscal[l] if fast_ln else 1.0
            nc.scalar.activation(
                lnv[:], var[:, :], AF.Ln,
                bias=eps_col[:, l: l + 1], scale=1.0 / (H * g * g),
            )
            sgs = psgs.tile([128, 512], BF, tag="sgs")
            nc.scalar.activation(
                sgs[:], lnv[:], AF.Exp, bias=zero_col[:, 0:1], scale=-0.5
            )
            for co in range(2):
                if fast_ln:
                    nc.vector.scalar_tensor_tensor(
                        h_t[:, co * HW + PAD + toff: co * HW + PAD + toff + 512],
                        zz[half][co][:, :], 0.0, sgs[:],
                        op0=OP.max, op1=OP.mult,
                    )
                else:
                    tmb = psgs.tile([128, 512], BF, tag="sgs", name="tmb")
                    nc.vector.scalar_tensor_tensor(
                        tmb[:], zz[half][co][:, :],
                        g_col[:, l * 2 + co: l * 2 + co + 1], sgs[:],
                        op0=OP.mult, op1=OP.mult,
                    )
                    nc.vector.tensor_scalar(
                        out=h_t[:, co * HW + PAD + toff
                                : co * HW + PAD + toff + 512],
                        in0=tmb[:],
                        scalar1=bb_sb[:, l * 2 + co: l * 2 + co + 1],
                        scalar2=0.0,
                        op0=OP.add, op1=OP.max,
                    )

        def layer_fin(st, l):
            st.pop("zz")
            st.pop("zsq")
            if l < 3:
                h_t = st["h"][l]
                hh_t = phh.tile([128, 2 * NN], BF, tag="hh")
                for co in range(2):
                    build_hh(
                        hh_t[:, co * NN:(co + 1) * NN],
                        h_t[:, co * HW + PAD: co * HW + PAD + NN],
                    )
                st["hh"] = hh_t

        def layer_round(pair, l):
            s0, s1 = pair
            layer_mm(s0, l)
            layer_ln_half(s0, l, 0)
            layer_mm(s1, l)
            layer_ln_half(s0, l, 1)
            layer_fin(s0, l)
            layer_ln_half(s1, l, 0)
            layer_ln_half(s1, l, 1)
            layer_fin(s1, l)

        def w1_chunk(st, m):
            if m == 0:
                st["zsb"] = pzsb.tile([128, 4096], BF, tag="zsb", name="zsb")
            zsb = st["zsb"]
            zw = [pz.tile([128, 512], FP, tag="z", name="zw") for _ in range(2)]
            for kc9 in range(9):
                if kc9 == 0:
                    lhsT = w1x_sb[:, m * 128:(m + 1) * 128]
                else:
                    j = kc9 - 1
                    lhsT = w1h_sb[:, j * 512 + m * 128: j * 512 + m * 128 + 128]
                for half in range(2):
                    toff = half * 512
                    if kc9 == 0:
                        rhs = st["x_fm"][:, PAD + toff: PAD + toff + 512]
                    else:
                        j = kc9 - 1
                        co = j % 2
                        rhs = st["h"][j // 2][:, co * HW + PAD + toff
                                              : co * HW + PAD + toff + 512]
                    nc.tensor.matmul(
                        zw[half][:, :], lhsT, rhs,
                        start=(kc9 == 0), stop=(kc9 == 8),
                    )
            for half in range(2):
                nc.scalar.activation(
                    zsb[:, m * NN + half * 512: m * NN + half * 512 + 512],
                    zw[half][:, :], AF.Relu,
                    scale=bns_sb[:, m: m + 1], bias=bnt_sb[:, m: m + 1],
                )

        def head_finish(st):
            s = st["s"]
            zsb = st["zsb"]
            y_s = pfin.tile([1, NN], FP, tag="ys")
            for c2 in range(2):
                yp = pvar.tile([1, 512], FP, tag="var", name="yp")
                for m in range(4):
                    nc.tensor.matmul(
                        yp[0:1, :], w2_sb[:, m: m + 1],
                        zsb[:, m * NN + c2 * 512: m * NN + c2 * 512 + 512],
                        start=(m == 0), stop=(m == 3),
                    )
                nc.vector.tensor_copy(y_s[:, c2 * 512:(c2 + 1) * 512], yp[0:1, :])
            if b2_val != 0.0:
                nc.scalar.add(y_s[:], y_s[:], b2_val)
            m_s = pfin.tile([1, NN], FP, tag="ms")
            nc.sync.dma_start(m_s[:], obs[s: s + 1, 0:NN])
            yf = pfin.tile([1, NN], FP, tag="yfin")
            nc.gpsimd.memset(yf[:], MIN_VAL)
            nc.vector.copy_predicated(yf[:], m_s[:].bitcast(mybir.dt.uint32), y_s[:])
            nc.sync.dma_start(y_out[s: s + 1, :], yf[:])

        def head_units(st):
            return [lambda m=m, st=st: w1_chunk(st, m) for m in range(4)] \
                + [lambda st=st: head_finish(st)]

        headq = []

        def flush(k=None):
            n = len(headq) if k is None else min(k, len(headq))
            for _ in range(n):
                headq.pop(0)()

        prev = [prep_x(0), prep_x(1)]
        for l in range(4):
            layer_round(prev, l)
        for p in range(1, 4):
            for st in prev:
                headq.extend(head_units(st))
            cur = [prep_x(2 * p), prep_x(2 * p + 1)]
            for l in range(4):
                layer_round(cur, l)
                flush(3)
            flush()
            prev = cur
        for st in prev:
            headq.extend(head_units(st))
        flush()

    nc.finalize()
    return nc


_BUILD_CACHE = {}
_CFG = None


def _get_nc(has_gin_bias=None, b2_val=None, use_bf16=None) -> bass.Bass:
    cfg = _CFG
    key = (cfg["has_gin_bias"], cfg["fast_ln"], cfg["gscal"], cfg["b2_val"])
    if key not in _BUILD_CACHE:
        _BUILD_CACHE[key] = _build(cfg)
    return _BUILD_CACHE[key]


def prep_maps(observations, W0, b0, g0, be0, Ws, bs, gs, bes,
              W1, b1, bn_g, bn_b, bn_m, bn_v, W2, b2, **_ignored):
    global _CFG
    obs = np.ascontiguousarray(np.asarray(observations, np.float32))
    W0 = np.asarray(W0, np.float32)
    Ws = np.asarray(Ws, np.float32)
    W1 = np.asarray(W1, np.float32)
    W2 = np.asarray(W2, np.float32)

    W0c = np.ascontiguousarray(W0 - W0.mean(axis=1, keepdims=True))
    Wsc = np.ascontiguousarray(Ws - Ws.mean(axis=2, keepdims=True))

    gg = np.stack([np.asarray(g0, np.float32)]
                  + [np.asarray(gs, np.float32)[i] for i in range(3)])
    bb = np.stack([np.asarray(be0, np.float32)]
                  + [np.asarray(bes, np.float32)[i] for i in range(3)])
    graw = np.stack([np.asarray(b0, np.float32)]
                    + [np.asarray(bs, np.float32)[i] for i in range(3)])
    gbias = np.ascontiguousarray(graw - graw.mean(axis=1, keepdims=True))
    has_gin_bias = bool(np.any(np.abs(gbias) > 0.0))

    g_const = all(np.all(gg[i] == gg[i][0]) for i in range(4))
    fast_ln = bool(np.all(bb == 0.0) and g_const
                   and all(gg[i][0] > 0 for i in range(4)))
    gscal = tuple(float(gg[i][0]) for i in range(4)) if fast_ln else (0.0,) * 4

    bn_scale = (np.asarray(bn_g, np.float32)
                / np.sqrt(np.asarray(bn_v, np.float32) + EPS_BN)).astype(np.float32)
    bn_shift = ((np.asarray(b1, np.float32) - np.asarray(bn_m, np.float32)) * bn_scale
                + np.asarray(bn_b, np.float32)).astype(np.float32)
    b2_val = float(np.asarray(b2, np.float32).reshape(-1)[0])

    w1x = np.ascontiguousarray(W1[:F_IN])
    w1h = np.ascontiguousarray(W1[F_IN:].reshape(8, 128, 512))
    w2r = np.ascontiguousarray(W2.reshape(4, 128))

    _CFG = {"has_gin_bias": has_gin_bias, "fast_ln": fast_ln,
            "gscal": gscal, "b2_val": b2_val}

    shared = {
        "w0": W0c, "ws": Wsc, "w1x": w1x, "w1h": w1h, "w2": w2r,
        "bns": bn_scale, "bnt": bn_shift,
    }
    if has_gin_bias:
        shared["gbias"] = gbias
    if not fast_ln:
        shared["grows"] = np.ascontiguousarray(gg)
        shared["bbr"] = np.ascontiguousarray(bb)
    in_maps = []
    for c in range(NCORE):
        m = dict(shared)
        m["obs"] = np.ascontiguousarray(obs[c * S: (c + 1) * S])
        in_maps.append(m)
    return in_maps, has_gin_bias, b2_val


def kernel(**inputs) -> np.ndarray:
    global LAST_EXEC_NS
    in_maps, has_gin_bias, b2_val = prep_maps(**inputs)
    nc = _get_nc(has_gin_bias, b2_val, USE_BF16)
    res = run_bass_kernel_spmd(
        nc, in_maps, list(range(NCORE)), trace=PROFILE, **TRACE_KWARGS
    )
    LAST_EXEC_NS = res.exec_time_ns
    y = np.concatenate([res.results[c]["y"] for c in range(NCORE)], axis=0)
    return y.reshape(B, NN).astype(np.float32)


# revision 30
# speedup vs baseline: 1.5981x; 1.3390x over previous
import numpy as np

import concourse.bass as bass
import concourse.bacc as bacc
import concourse.mybir as mybir
import concourse.tile as tile
from concourse.bass_utils import run_bass_kernel_spmd
from concourse.masks import make_identity

FP = mybir.dt.float32
BF = mybir.dt.bfloat16
U16 = mybir.dt.uint16
AF = mybir.ActivationFunctionType
OP = mybir.AluOpType

GRID = 32
NN = 1024
F_IN = 32
H = 256
B = 64
S = 8
NCORE = 8
OBS_W = NN + NN * F_IN
MIN_VAL = -10000000.0
EPS_LN = 1e-5
EPS_BN = 1e-5
PAD = 32
HW = NN + 2 * PAD

USE_BF16 = True
PROFILE = False
LAST_EXEC_NS = None
TRACE_KWARGS = {}

_ACT_PATCHED = False


def _patch_act_tables():
    global _ACT_PATCHED
    if _ACT_PATCHED:
        return
    import concourse.hw_specs as hw_specs

    orig = hw_specs.get_activation_tables

    def patched(arch):
        t = orig(arch)
        if "natural_log_exp_and_others" in t:
            both = t["natural_log_exp_and_others"]
            if AF.Ln in both and AF.Exp in both:
                for name, fns in t.items():
                    if name != "natural_log_exp_and_others":
                        fns.discard(AF.Exp)
                        fns.discard(AF.Ln)
        return t

    hw_specs.get_activation_tables = patched
    for mod in (bacc, bass):
        if getattr(mod, "get_activation_tables", None) is orig:
            mod.get_activation_tables = patched
    _ACT_PATCHED = True


def _build(cfg, unused_b2=None, unused_bf16=None) -> bass.Bass:
    has_gin_bias = cfg["has_gin_bias"]
    fast_ln = cfg["fast_ln"]
    gscal = cfg["gscal"]
    b2_val = cfg["b2_val"]

    _patch_act_tables()
    nc = bacc.Bacc("TRN2", target_bir_lowering=False, debug=False)

    obs = nc.declare_dram_parameter("obs", [S, OBS_W], FP, isOutput=False)
    w0 = nc.declare_dram_parameter("w0", [96, H], U16, isOutput=False)
    ws = nc.declare_dram_parameter("ws", [3, H, H], U16, isOutput=False)
    w1x = nc.declare_dram_parameter("w1x", [F_IN, 512], U16, isOutput=False)
    w1h = nc.declare_dram_parameter("w1h", [8, 128, 512], U16, isOutput=False)
    w2 = nc.declare_dram_parameter("w2", [4, 128], U16, isOutput=False)
    bns = nc.declare_dram_parameter("bns", [512], FP, isOutput=False)
    bnt = nc.declare_dram_parameter("bnt", [512], FP, isOutput=False)
    if has_gin_bias:
        gbias = nc.declare_dram_parameter("gbias", [4, H], U16, isOutput=False)
    if not fast_ln:
        grows = nc.declare_dram_parameter("grows", [4, H], FP, isOutput=False)
        bbr = nc.declare_dram_parameter("bbr", [4, H], FP, isOutput=False)
    y_out = nc.declare_dram_parameter("y", [S, NN], FP, isOutput=True)

    from contextlib import ExitStack

    with tile.TileContext(nc) as tc, ExitStack() as ctx:
        wp = ctx.enter_context(tc.tile_pool(name="w", bufs=1))
        px = ctx.enter_context(tc.tile_pool(name="px", bufs=2))
        pxs = ctx.enter_context(tc.tile_pool(name="pxs", bufs=9))
        ph = ctx.enter_context(tc.tile_pool(name="ph", bufs=17))
        phh = ctx.enter_context(tc.tile_pool(name="phh", bufs=3))
        pzsq = ctx.enter_context(tc.tile_pool(name="pzsq", bufs=3))
        plnv = ctx.enter_context(tc.tile_pool(name="plnv", bufs=3))
        psgs = ctx.enter_context(tc.tile_pool(name="psgs", bufs=3))
        pzsb = ctx.enter_context(tc.tile_pool(name="pzsb", bufs=2))
        pfin = ctx.enter_context(tc.tile_pool(name="pfin", bufs=2))
        pz = ctx.enter_context(tc.tile_pool(name="pz", bufs=6, space="PSUM"))
        pvar = ctx.enter_context(tc.tile_pool(name="pvar", bufs=2, space="PSUM"))

        ident = wp.tile([128, 128], FP, tag="id")
        make_identity(nc, ident[:])

        ones_mat = wp.tile([128, 128], BF, tag="ones_mat")
        nc.gpsimd.memset(ones_mat[:].bitcast(U16), 0x3F80)
        ones_row = wp.tile([1, 512], BF, tag="ones_row")
        nc.gpsimd.memset(ones_row[:].bitcast(U16), 0x3F80)

        zero_col = wp.tile([128, 1], FP, tag="zero_col")
        nc.gpsimd.memset(zero_col[:], 0.0)
        eps_col = wp.tile([128, 4], FP, tag="eps_col")
        for l in range(4):
            gl = gscal[l] if fast_ln else 1.0
            nc.gpsimd.memset(eps_col[:, l: l + 1], EPS_LN / (gl * gl))

        w0_sb = wp.tile([96, H], BF, tag="w0")
        nc.scalar.dma_start(w0_sb[:].bitcast(U16), w0[:, :])

        wl_sb = wp.tile([128, 3 * 512], BF, tag="wl")
        for l in range(3):
            nc.scalar.dma_start(
                wl_sb[:, l * 512:(l + 1) * 512].bitcast(U16)
                .rearrange("p (ci co c) -> p ci co c", ci=2, co=2),
                ws[l].rearrange("(ci p) (co c) -> p ci co c", p=128, c=128),
            )

        w1x_sb = wp.tile([F_IN, 512], BF, tag="w1x")
        nc.scalar.dma_start(w1x_sb[:, :].bitcast(U16), w1x[:, :])
        w1h_sb = wp.tile([128, 8 * 512], BF, tag="w1h")
        nc.scalar.dma_start(
            w1h_sb[:].bitcast(U16).rearrange("p (j m) -> p j m", j=8),
            w1h[:, :, :].rearrange("j p m -> p j m"),
        )
        w2_sb = wp.tile([128, 4], BF, tag="w2")
        nc.scalar.dma_start(w2_sb[:].bitcast(U16), w2[:, :].rearrange("k p -> p k"))

        bns_sb = wp.tile([128, 4], FP, tag="bns")
        nc.scalar.dma_start(bns_sb[:], bns[:].rearrange("(m p) -> p m", p=128))
        bnt_sb = wp.tile([128, 4], FP, tag="bnt")
        nc.scalar.dma_start(bnt_sb[:], bnt[:].rearrange("(m p) -> p m", p=128))

        if has_gin_bias:
            gb_row = wp.tile([1, 4 * H], BF, tag="gb")
            nc.scalar.dma_start(
                gb_row[:].bitcast(U16).rearrange("q (l n) -> q l n", l=4),
                gbias[:, :],
            )
        if not fast_ln:
            g_col = wp.tile([128, 8], FP, tag="gcol")
            nc.scalar.dma_start(
                g_col[:].rearrange("p (l c) -> p l c", c=2),
                grows[:, :].rearrange("l (c p) -> p l c", p=128),
            )
            bb_sb = wp.tile([128, 8], FP, tag="bb")
            nc.scalar.dma_start(
                bb_sb[:].rearrange("p (l c) -> p l c", c=2),
                bbr[:, :].rearrange("l (c p) -> p l c", p=128),
            )

        def wchunk(l, ci, co):
            if l == 0:
                return w0_sb[:, co * 128: co * 128 + 128]
            return wl_sb[:, (l - 1) * 512 + ci * 256 + co * 128
                         : (l - 1) * 512 + ci * 256 + co * 128 + 128]

        def build_hh(hh_ap, src_ap):
            sv = src_ap.rearrange("p (r c) -> p r c", c=GRID)
            dv = hh_ap.rearrange("p (r c) -> p r c", c=GRID)
            nc.gpsimd.tensor_add(dv[:, :, 1:31], sv[:, :, 0:30], sv[:, :, 2:32])
            nc.gpsimd.tensor_copy(dv[:, :, 0:1], sv[:, :, 1:2])
            nc.gpsimd.tensor_copy(dv[:, :, 31:32], sv[:, :, 30:31])

        def prep_x(s):
            x_nm = px.tile([128, 256], FP, tag="xnm")
            for hb in range(2):
                nc.scalar.dma_start(
                    x_nm[:, hb * 128:(hb + 1) * 128]
                    .rearrange("p (b f) -> p b f", f=F_IN),
                    obs[s, NN + hb * 4096 * 4: NN + (hb + 1) * 4096 * 4]
                    .rearrange("(b p f) -> p b f", p=128, f=F_IN),
                )
            xs = pxs.tile([96, HW], BF, tag="xs")
            nc.gpsimd.memset(xs[0:32, 32: 64].bitcast(U16), 0)
            nc.gpsimd.memset(xs[64:96, NN: NN + 32].bitcast(U16), 0)
            for half in range(2):
                x_tfm = pz.tile([F_IN, 512], FP, tag="z", name="xtf")
                for i in range(4):
                    b = half * 4 + i
                    nc.tensor.transpose(
                        x_tfm[:, i * 128:(i + 1) * 128],
                        x_nm[:, b * F_IN:(b + 1) * F_IN],
                        ident[:],
                    )
                nc.vector.tensor_copy(
                    xs[0:32, 2 * PAD + half * 512: 2 * PAD + half * 512 + 512],
                    x_tfm[:],
                )
                nc.vector.tensor_copy(
                    xs[64:96, half * 512: half * 512 + 512], x_tfm[:]
                )
            build_hh(xs[32:64, PAD: PAD + NN], xs[0:32, 2 * PAD: 2 * PAD + NN])
            return {"s": s, "xs": xs, "h": []}

        def layer_mm(st, l):
            kc = 1 if l == 0 else 2
            st["zsq"] = pzsq.tile([128, 2048], BF, tag="zsq", name="zsq")
            h_t = ph.tile([128, 2 * HW], BF, tag="ht", name="ht")
            nc.gpsimd.memset(h_t[:, 0:PAD].bitcast(U16), 0)
            nc.gpsimd.memset(h_t[:, PAD + NN: HW + PAD].bitcast(U16), 0)
            nc.gpsimd.memset(h_t[:, HW + PAD + NN: 2 * HW].bitcast(U16), 0)
            st["h"].append(h_t)
            zz = [[None, None], [None, None]]
            for half in range(2):
                toff = half * 512
                for co in range(2):
                    z = pz.tile([128, 512], FP, tag="z")
                    if l == 0:
                        nmm = 1 + (1 if has_gin_bias else 0)
                        nc.tensor.matmul(
                            z[:, :], wchunk(0, 0, co),
                            st["xs"][:, PAD + toff: PAD + toff + 512],
                            start=True, stop=(nmm == 1),
                        )
                    else:
                        nmm = 3 * kc + (1 if has_gin_bias else 0)
                        i = 0
                        for ci in range(kc):
                            hprev = st["h"][l - 1]
                            hhprev = st["hh"]
                            hh_w = hhprev[:, ci * NN + toff: ci * NN + toff + 512]
                            up_w = hprev[:, ci * HW + toff: ci * HW + toff + 512]
                            dn_w = hprev[:, ci * HW + 2 * PAD + toff
                                         : ci * HW + 2 * PAD + toff + 512]
                            for rhs in (hh_w, up_w, dn_w):
                                nc.tensor.matmul(
                                    z[:, :], wchunk(l, ci, co), rhs,
                                    start=(i == 0), stop=(i == nmm - 1),
                                )
                                i += 1
                    if has_gin_bias:
                        nc.tensor.matmul(
                            z[:, :],
                            gb_row[0:1, l * H + co * 128: l * H + co * 128 + 128],
                            ones_row[0:1, 0:512],
                            start=False, stop=True,
                        )
                    zz[half][co] = z
                for co in range(2):
                    nc.scalar.activation(
                        st["zsq"][:, co * NN + toff: co * NN + toff + 512],
                        zz[half][co][:, :], AF.Square, bias=zero_col[:, 0:1],
                    )
            st["zz"] = zz

        def layer_ln_half(st, l, half):
            toff = half * 512
            zz = st["zz"]
            zsq = st["zsq"]
            h_t = st["h"][l]
            var = pvar.tile([128, 512], FP, tag="var")
            for co in range(2):
                nc.tensor.matmul(
                    var[:, :], ones_mat[:, :],
                    zsq[:, co * NN + toff: co * NN + toff + 512],
                    start=(co == 0), stop=(co == 1),
                )
            lnv = plnv.tile([128, 512], FP, tag="lnv")
            g = gscal[l] if fast_ln else 1.0
            nc.scalar.activation(
                lnv[:], var[:, :], AF.Ln,
                bias=eps_col[:, l: l + 1], scale=1.0 / (H * g * g),
            )
            sgs = psgs.tile([128, 512], BF, tag="sgs")
            nc.scalar.activation(
                sgs[:], lnv[:], AF.Exp, bias=zero_col[:, 0:1], scale=-0.5
            )
            for co in range(2):
                if fast_ln:
                    nc.vector.scalar_tensor_tensor(
                        h_t[:, co * HW + PAD + toff: co * HW + PAD + toff + 512],
                        zz[half][co][:, :], 0.0, sgs[:],
                        op0=OP.max, op1=OP.mult,
                    )
                else:
                    tmb = psgs.tile([128, 512], BF, tag="sgs", name="tmb")
                    nc.vector.scalar_tensor_tensor(
                        tmb[:], zz[half][co][:, :],
                        g_col[:, l * 2 + co: l * 2 + co + 1], sgs[:],
                        op0=OP.mult, op1=OP.mult,
                    )
                    nc.vector.tensor_scalar(
                        out=h_t[:, co * HW + PAD + toff
                                : co * HW + PAD + toff + 512],
                        in0=tmb[:],
                        scalar1=bb_sb[:, l * 2 + co: l * 2 + co + 1],
                        scalar2=0.0,
                        op0=OP.add, op1=OP.max,
                    )

        def layer_fin(st, l):
            st.pop("zz")
            st.pop("zsq")
            if l < 3:
                h_t = st["h"][l]
                hh_t = phh.tile([128, 2 * NN], BF, tag="hh")
                for co in range(2):
                    build_hh(
                        hh_t[:, co * NN:(co + 1) * NN],
                        h_t[:, co * HW + PAD: co * HW + PAD + NN],
                    )
                st["hh"] = hh_t

        def layer_round(pair, l):
            s0, s1 = pair
            layer_mm(s0, l)
            layer_ln_half(s0, l, 0)
            layer_mm(s1, l)
            layer_ln_half(s0, l, 1)
            layer_fin(s0, l)
            layer_ln_half(s1, l, 0)
            layer_ln_half(s1, l, 1)
            layer_fin(s1, l)

        def w1_chunk(st, m):
            if m == 0:
                st["zsb"] = pzsb.tile([128, 4096], BF, tag="zsb", name="zsb")
            zsb = st["zsb"]
            zw = [pz.tile([128, 512], FP, tag="z", name="zw") for _ in range(2)]
            for kc9 in range(9):
                if kc9 == 0:
                    lhsT = w1x_sb[:, m * 128:(m + 1) * 128]
                else:
                    j = kc9 - 1
                    lhsT = w1h_sb[:, j * 512 + m * 128: j * 512 + m * 128 + 128]
                for half in range(2):
                    toff = half * 512
                    if kc9 == 0:
                        rhs = st["xs"][0:32, 2 * PAD + toff: 2 * PAD + toff + 512]
                    else:
                        j = kc9 - 1
                        co = j % 2
                        rhs = st["h"][j // 2][:, co * HW + PAD + toff
                                              : co * HW + PAD + toff + 512]
                    nc.tensor.matmul(
                        zw[half][:, :], lhsT, rhs,
                        start=(kc9 == 0), stop=(kc9 == 8),
                    )
            for half in range(2):
                nc.scalar.activation(
                    zsb[:, m * NN + half * 512: m * NN + half * 512 + 512],
                    zw[half][:, :], AF.Relu,
                    scale=bns_sb[:, m: m + 1], bias=bnt_sb[:, m: m + 1],
                )

        def head_finish(st):
            s = st["s"]
            zsb = st["zsb"]
            y_s = pfin.tile([1, NN], FP, tag="ys")
            for c2 in range(2):
                yp = pvar.tile([1, 512], FP, tag="var", name="yp")
                for m in range(4):
                    nc.tensor.matmul(
                        yp[0:1, :], w2_sb[:, m: m + 1],
                        zsb[:, m * NN + c2 * 512: m * NN + c2 * 512 + 512],
                        start=(m == 0), stop=(m == 3),
                    )
                nc.vector.tensor_copy(y_s[:, c2 * 512:(c2 + 1) * 512], yp[0:1, :])
            if b2_val != 0.0:
                nc.scalar.add(y_s[:], y_s[:], b2_val)
            m_s = pfin.tile([1, NN], FP, tag="ms")
            nc.sync.dma_start(m_s[:], obs[s: s + 1, 0:NN])
            yf = pfin.tile([1, NN], FP, tag="yfin")
            nc.gpsimd.memset(yf[:], MIN_VAL)
            nc.vector.copy_predicated(yf[:], m_s[:].bitcast(mybir.dt.uint32), y_s[:])
            nc.sync.dma_start(y_out[s: s + 1, :], yf[:])

        def head_units(st):
            return [lambda m=m, st=st: w1_chunk(st, m) for m in range(4)] \
                + [lambda st=st: head_finish(st)]

        headq = []

        def flush(k=None):
            n = len(headq) if k is None else min(k, len(headq))
            for _ in range(n):
                headq.pop(0)()

        prev = [prep_x(0), prep_x(1)]
        for l in range(4):
            layer_round(prev, l)
        for p in range(1, 4):
            for st in prev:
                headq.extend(head_units(st))
            cur = [prep_x(2 * p), prep_x(2 * p + 1)]
            for l in range(4):
                layer_round(cur, l)
                flush(3)
            flush()
            prev = cur
        for st in prev:
            headq.extend(head_units(st))
        flush()

    nc.finalize()
    return nc


_BUILD_CACHE = {}
_CFG = None


def _get_nc(has_gin_bias=None, b2_val=None, use_bf16=None) -> bass.Bass:
    cfg = _CFG
    key = (cfg["has_gin_bias"], cfg["fast_ln"], cfg["gscal"], cfg["b2_val"])
    if key not in _BUILD_CACHE:
        _BUILD_CACHE[key] = _build(cfg)
    return _BUILD_CACHE[key]


def prep_maps(observations, W0, b0, g0, be0, Ws, bs, gs, bes,
              W1, b1, bn_g, bn_b, bn_m, bn_v, W2, b2, **_ignored):
    global _CFG
    obs = np.ascontiguousarray(np.asarray(observations, np.float32))
    W0 = np.asarray(W0, np.float32)
    Ws = np.asarray(Ws, np.float32)
    W1 = np.asarray(W1, np.float32)
    W2 = np.asarray(W2, np.float32)

    W0c = np.ascontiguousarray(W0 - W0.mean(axis=1, keepdims=True))
    Wsc = np.ascontiguousarray(Ws - Ws.mean(axis=2, keepdims=True))

    gg = np.stack([np.asarray(g0, np.float32)]
                  + [np.asarray(gs, np.float32)[i] for i in range(3)])
    bb = np.stack([np.asarray(be0, np.float32)]
                  + [np.asarray(bes, np.float32)[i] for i in range(3)])
    graw = np.stack([np.asarray(b0, np.float32)]
                    + [np.asarray(bs, np.float32)[i] for i in range(3)])
    gbias = np.ascontiguousarray(graw - graw.mean(axis=1, keepdims=True))
    has_gin_bias = bool(np.any(np.abs(gbias) > 0.0))

    g_const = all(np.all(gg[i] == gg[i][0]) for i in range(4))
    fast_ln = bool(np.all(bb == 0.0) and g_const
                   and all(gg[i][0] > 0 for i in range(4)))
    gscal = tuple(float(gg[i][0]) for i in range(4)) if fast_ln else (0.0,) * 4

    bn_scale = (np.asarray(bn_g, np.float32)
                / np.sqrt(np.asarray(bn_v, np.float32) + EPS_BN)).astype(np.float32)
    bn_shift = ((np.asarray(b1, np.float32) - np.asarray(bn_m, np.float32)) * bn_scale
                + np.asarray(bn_b, np.float32)).astype(np.float32)
    b2_val = float(np.asarray(b2, np.float32).reshape(-1)[0])

    def bf16(a):
        t = np.ascontiguousarray(a, np.float32).view(np.uint32)
        r = ((t + 0x7FFF + ((t >> 16) & 1)) >> 16).astype(np.uint16)
        return r

    w1x = np.ascontiguousarray(W1[:F_IN])
    w1h = np.ascontiguousarray(W1[F_IN:].reshape(8, 128, 512))
    w2r = np.ascontiguousarray(W2.reshape(4, 128))

    _CFG = {"has_gin_bias": has_gin_bias, "fast_ln": fast_ln,
            "gscal": gscal, "b2_val": b2_val}

    shared = {
        "w0": bf16(W0c), "ws": bf16(Wsc), "w1x": bf16(w1x),
        "w1h": bf16(w1h), "w2": bf16(w2r),
        "bns": bn_scale, "bnt": bn_shift,
    }
    if has_gin_bias:
        shared["gbias"] = bf16(gbias)
    if not fast_ln:
        shared["grows"] = np.ascontiguousarray(gg)
        shared["bbr"] = np.ascontiguousarray(bb)
    in_maps = []
    for c in range(NCORE):
        m = dict(shared)
        m["obs"] = np.ascontiguousarray(obs[c * S: (c + 1) * S])
        in_maps.append(m)
    return in_maps, has_gin_bias, b2_val


def kernel(**inputs) -> np.ndarray:
    global LAST_EXEC_NS
    in_maps, has_gin_bias, b2_val = prep_maps(**inputs)
    nc = _get_nc(has_gin_bias, b2_val, USE_BF16)
    res = run_bass_kernel_spmd(
        nc, in_maps, list(range(NCORE)), trace=PROFILE, **TRACE_KWARGS
    )
    LAST_EXEC_NS = res.exec_time_ns
    y = np.concatenate([res.results[c]["y"] for c in range(NCORE)], axis=0)
    return y.reshape(B, NN).astype(np.float32)


# revision 34
# speedup vs baseline: 1.6418x; 1.0273x over previous
import numpy as np

import concourse.bass as bass
import concourse.bacc as bacc
import concourse.mybir as mybir
import concourse.tile as tile
from concourse.bass_utils import run_bass_kernel_spmd
from concourse.masks import make_identity

FP = mybir.dt.float32
BF = mybir.dt.bfloat16
U16 = mybir.dt.uint16
AF = mybir.ActivationFunctionType
OP = mybir.AluOpType

GRID = 32
NN = 1024
F_IN = 32
H = 256
B = 64
S = 8
NCORE = 8
OBS_W = NN + NN * F_IN
MIN_VAL = -10000000.0
EPS_LN = 1e-5
EPS_BN = 1e-5
PAD = 32
HW = NN + 2 * PAD

USE_BF16 = True
PROFILE = False
LAST_EXEC_NS = None
TRACE_KWARGS = {}

_ACT_PATCHED = False


def _patch_act_tables():
    global _ACT_PATCHED
    if _ACT_PATCHED:
        return
    import concourse.hw_specs as hw_specs

    orig = hw_specs.get_activation_tables

    def patched(arch):
        t = orig(arch)
        if "natural_log_exp_and_others" in t:
            both = t["natural_log_exp_and_others"]
            if AF.Ln in both and AF.Exp in both:
                for name, fns in t.items():
                    if name != "natural_log_exp_and_others":
                        fns.discard(AF.Exp)
                        fns.discard(AF.Ln)
        return t

    hw_specs.get_activation_tables = patched
    for mod in (bacc, bass):
        if getattr(mod, "get_activation_tables", None) is orig:
            mod.get_activation_tables = patched
    _ACT_PATCHED = True


def _build(cfg, unused_b2=None, unused_bf16=None) -> bass.Bass:
    has_gin_bias = cfg["has_gin_bias"]
    fast_ln = cfg["fast_ln"]
    gscal = cfg["gscal"]
    b2_val = cfg["b2_val"]

    _patch_act_tables()
    nc = bacc.Bacc("TRN2", target_bir_lowering=False, debug=False)

    obs = nc.declare_dram_parameter("obs", [S, OBS_W], FP, isOutput=False)
    w0 = nc.declare_dram_parameter("w0", [96, H], U16, isOutput=False)
    ws = nc.declare_dram_parameter("ws", [3, H, H], U16, isOutput=False)
    w1x = nc.declare_dram_parameter("w1x", [F_IN, 512], U16, isOutput=False)
    w1h = nc.declare_dram_parameter("w1h", [8, 128, 512], U16, isOutput=False)
    w2 = nc.declare_dram_parameter("w2", [4, 128], U16, isOutput=False)
    bns = nc.declare_dram_parameter("bns", [512], FP, isOutput=False)
    bnt = nc.declare_dram_parameter("bnt", [512], FP, isOutput=False)
    if has_gin_bias:
        gbias = nc.declare_dram_parameter("gbias", [4, H], U16, isOutput=False)
    if not fast_ln:
        grows = nc.declare_dram_parameter("grows", [4, H], FP, isOutput=False)
        bbr = nc.declare_dram_parameter("bbr", [4, H], FP, isOutput=False)
    y_out = nc.declare_dram_parameter("y", [S, NN], FP, isOutput=True)

    from contextlib import ExitStack

    with tile.TileContext(nc) as tc, ExitStack() as ctx:
        wp = ctx.enter_context(tc.tile_pool(name="w", bufs=1))
        px = ctx.enter_context(tc.tile_pool(name="px", bufs=2))
        pxs = ctx.enter_context(tc.tile_pool(name="pxs", bufs=9))
        ph = ctx.enter_context(tc.tile_pool(name="ph", bufs=17))
        phh = ctx.enter_context(tc.tile_pool(name="phh", bufs=3))
        pzsq = ctx.enter_context(tc.tile_pool(name="pzsq", bufs=3))
        plnv = ctx.enter_context(tc.tile_pool(name="plnv", bufs=3))
        psgs = ctx.enter_context(tc.tile_pool(name="psgs", bufs=3))
        pzsb = ctx.enter_context(tc.tile_pool(name="pzsb", bufs=2))
        pfin = ctx.enter_context(tc.tile_pool(name="pfin", bufs=2))
        pz = ctx.enter_context(tc.tile_pool(name="pz", bufs=6, space="PSUM"))
        pvar = ctx.enter_context(tc.tile_pool(name="pvar", bufs=2, space="PSUM"))

        ident = wp.tile([128, 128], FP, tag="id")
        make_identity(nc, ident[:])

        ones_mat = wp.tile([128, 128], BF, tag="ones_mat")
        nc.gpsimd.memset(ones_mat[:].bitcast(U16), 0x3F80)
        ones_row = wp.tile([1, 512], BF, tag="ones_row")
        nc.gpsimd.memset(ones_row[:].bitcast(U16), 0x3F80)

        zero_col = wp.tile([128, 1], FP, tag="zero_col")
        nc.gpsimd.memset(zero_col[:], 0.0)
        eps_col = wp.tile([128, 4], FP, tag="eps_col")
        for l in range(4):
            gl = gscal[l] if fast_ln else 1.0
            nc.gpsimd.memset(eps_col[:, l: l + 1], EPS_LN / (gl * gl))

        w0_sb = wp.tile([96, H], BF, tag="w0")
        nc.scalar.dma_start(w0_sb[:].bitcast(U16), w0[:, :])

        wl_sb = wp.tile([128, 3 * 512], BF, tag="wl")
        for l in range(3):
            nc.scalar.dma_start(
                wl_sb[:, l * 512:(l + 1) * 512].bitcast(U16)
                .rearrange("p (ci co c) -> p ci co c", ci=2, co=2),
                ws[l].rearrange("(ci p) (co c) -> p ci co c", p=128, c=128),
            )

        w1x_sb = wp.tile([F_IN, 512], BF, tag="w1x")
        nc.scalar.dma_start(w1x_sb[:, :].bitcast(U16), w1x[:, :])
        w1h_sb = wp.tile([128, 8 * 512], BF, tag="w1h")
        nc.scalar.dma_start(
            w1h_sb[:].bitcast(U16).rearrange("p (j m) -> p j m", j=8),
            w1h[:, :, :].rearrange("j p m -> p j m"),
        )
        w2_sb = wp.tile([128, 4], BF, tag="w2")
        nc.scalar.dma_start(w2_sb[:].bitcast(U16), w2[:, :].rearrange("k p -> p k"))

        bns_sb = wp.tile([128, 4], FP, tag="bns")
        nc.scalar.dma_start(bns_sb[:], bns[:].rearrange("(m p) -> p m", p=128))
        bnt_sb = wp.tile([128, 4], FP, tag="bnt")
        nc.scalar.dma_start(bnt_sb[:], bnt[:].rearrange("(m p) -> p m", p=128))

        if has_gin_bias:
            gb_row = wp.tile([1, 4 * H], BF, tag="gb")
            nc.scalar.dma_start(
                gb_row[:].bitcast(U16).rearrange("q (l n) -> q l n", l=4),
                gbias[:, :],
            )
        if not fast_ln:
            g_col = wp.tile([128, 8], FP, tag="gcol")
            nc.scalar.dma_start(
                g_col[:].rearrange("p (l c) -> p l c", c=2),
                grows[:, :].rearrange("l (c p) -> p l c", p=128),
            )
            bb_sb = wp.tile([128, 8], FP, tag="bb")
            nc.scalar.dma_start(
                bb_sb[:].rearrange("p (l c) -> p l c", c=2),
                bbr[:, :].rearrange("l (c p) -> p l c", p=128),
            )

        warm_ps = pz.tile([128, 128], FP, tag="z", name="warm_ps")
        for i in range(30):
            nc.tensor.matmul(
                warm_ps[:, :], ones_mat[:, :], ones_mat[:, :],
                start=(i == 0), stop=(i == 29),
            )

        def wchunk(l, ci, co):
            if l == 0:
                return w0_sb[:, co * 128: co * 128 + 128]
            return wl_sb[:, (l - 1) * 512 + ci * 256 + co * 128
                         : (l - 1) * 512 + ci * 256 + co * 128 + 128]

        def build_hh(hh_ap, src_ap):
            sv = src_ap.rearrange("p (r c) -> p r c", c=GRID)
            dv = hh_ap.rearrange("p (r c) -> p r c", c=GRID)
            nc.gpsimd.tensor_add(dv[:, :, 1:31], sv[:, :, 0:30], sv[:, :, 2:32])
            nc.gpsimd.tensor_copy(dv[:, :, 0:1], sv[:, :, 1:2])
            nc.gpsimd.tensor_copy(dv[:, :, 31:32], sv[:, :, 30:31])

        def prep_x(s):
            x_nm = px.tile([128, 256], FP, tag="xnm")
            nc.sync.dma_start(
                x_nm[:].rearrange("p (b f) -> p b f", f=F_IN),
                obs[s, NN:OBS_W].rearrange("(b p f) -> p b f", p=128, f=F_IN),
            )
            xs = pxs.tile([96, HW], BF, tag="xs")
            nc.gpsimd.memset(xs[0:32, 32: 64].bitcast(U16), 0)
            nc.gpsimd.memset(xs[64:96, NN: NN + 32].bitcast(U16), 0)
            for half in range(2):
                x_tfm = pz.tile([F_IN, 512], FP, tag="z", name="xtf")
                for i in range(4):
                    b = half * 4 + i
                    nc.tensor.transpose(
                        x_tfm[:, i * 128:(i + 1) * 128],
                        x_nm[:, b * F_IN:(b + 1) * F_IN],
                        ident[:],
                    )
                nc.vector.tensor_copy(
                    xs[0:32, 2 * PAD + half * 512: 2 * PAD + half * 512 + 512],
                    x_tfm[:],
                )
                nc.vector.tensor_copy(
                    xs[64:96, half * 512: half * 512 + 512], x_tfm[:]
                )
            build_hh(xs[32:64, PAD: PAD + NN], xs[0:32, 2 * PAD: 2 * PAD + NN])
            return {"s": s, "xs": xs, "h": []}

        def layer_mm(st, l):
            kc = 1 if l == 0 else 2
            st["zsq"] = pzsq.tile([128, 2048], BF, tag="zsq", name="zsq")
            h_t = ph.tile([128, 2 * HW], BF, tag="ht", name="ht")
            nc.gpsimd.memset(h_t[:, 0:PAD].bitcast(U16), 0)
            nc.gpsimd.memset(h_t[:, PAD + NN: HW + PAD].bitcast(U16), 0)
            nc.gpsimd.memset(h_t[:, HW + PAD + NN: 2 * HW].bitcast(U16), 0)
            st["h"].append(h_t)
            zz = [[None, None], [None, None]]
            for half in range(2):
                toff = half * 512
                for co in range(2):
                    z = pz.tile([128, 512], FP, tag="z")
                    if l == 0:
                        nmm = 1 + (1 if has_gin_bias else 0)
                        nc.tensor.matmul(
                            z[:, :], wchunk(0, 0, co),
                            st["xs"][:, PAD + toff: PAD + toff + 512],
                            start=True, stop=(nmm == 1),
                        )
                    else:
                        nmm = 3 * kc + (1 if has_gin_bias else 0)
                        i = 0
                        for ci in range(kc):
                            hprev = st["h"][l - 1]
                            hhprev = st["hh"]
                            hh_w = hhprev[:, ci * NN + toff: ci * NN + toff + 512]
                            up_w = hprev[:, ci * HW + toff: ci * HW + toff + 512]
                            dn_w = hprev[:, ci * HW + 2 * PAD + toff
                                         : ci * HW + 2 * PAD + toff + 512]
                            for rhs in (hh_w, up_w, dn_w):
                                nc.tensor.matmul(
                                    z[:, :], wchunk(l, ci, co), rhs,
                                    start=(i == 0), stop=(i == nmm - 1),
                                )
                                i += 1
                    if has_gin_bias:
                        nc.tensor.matmul(
                            z[:, :],
                            gb_row[0:1, l * H + co * 128: l * H + co * 128 + 128],
                            ones_row[0:1, 0:512],
                            start=False, stop=True,
                        )
                    zz[half][co] = z
                for co in range(2):
                    nc.scalar.activation(
                        st["zsq"][:, co * NN + toff: co * NN + toff + 512],
                        zz[half][co][:, :], AF.Square, bias=zero_col[:, 0:1],
                    )
            st["zz"] = zz

        def layer_ln_half(st, l, half):
            toff = half * 512
            zz = st["zz"]
            zsq = st["zsq"]
            h_t = st["h"][l]
            var = pvar.tile([128, 512], FP, tag="var")
            for co in range(2):
                nc.tensor.matmul(
                    var[:, :], ones_mat[:, :],
                    zsq[:, co * NN + toff: co * NN + toff + 512],
                    start=(co == 0), stop=(co == 1),
                )
            lnv = plnv.tile([128, 512], FP, tag="lnv")
            g = gscal[l] if fast_ln else 1.0
            nc.scalar.activation(
                lnv[:], var[:, :], AF.Ln,
                bias=eps_col[:, l: l + 1], scale=1.0 / (H * g * g),
            )
            sgs = psgs.tile([128, 512], BF, tag="sgs")
            nc.scalar.activation(
                sgs[:], lnv[:], AF.Exp, bias=zero_col[:, 0:1], scale=-0.5
            )
            for co in range(2):
                if fast_ln:
                    nc.vector.scalar_tensor_tensor(
                        h_t[:, co * HW + PAD + toff: co * HW + PAD + toff + 512],
                        zz[half][co][:, :], 0.0, sgs[:],
                        op0=OP.max, op1=OP.mult,
                    )
                else:
                    tmb = psgs.tile([128, 512], BF, tag="sgs", name="tmb")
                    nc.vector.scalar_tensor_tensor(
                        tmb[:], zz[half][co][:, :],
                        g_col[:, l * 2 + co: l * 2 + co + 1], sgs[:],
                        op0=OP.mult, op1=OP.mult,
                    )
                    nc.vector.tensor_scalar(
                        out=h_t[:, co * HW + PAD + toff
                                : co * HW + PAD + toff + 512],
                        in0=tmb[:],
                        scalar1=bb_sb[:, l * 2 + co: l * 2 + co + 1],
                        scalar2=0.0,
                        op0=OP.add, op1=OP.max,
                    )

        def layer_fin(st, l):
            st.pop("zz")
            st.pop("zsq")
            if l < 3:
                h_t = st["h"][l]
                hh_t = phh.tile([128, 2 * NN], BF, tag="hh")
                for co in range(2):
                    build_hh(
                        hh_t[:, co * NN:(co + 1) * NN],
                        h_t[:, co * HW + PAD: co * HW + PAD + NN],
                    )
                st["hh"] = hh_t

        def layer_round(pair, l):
            s0, s1 = pair
            layer_mm(s0, l)
            layer_ln_half(s0, l, 0)
            layer_mm(s1, l)
            layer_ln_half(s0, l, 1)
            layer_fin(s0, l)
            layer_ln_half(s1, l, 0)
            layer_ln_half(s1, l, 1)
            layer_fin(s1, l)

        def w1_chunk(st, m):
            if m == 0:
                st["zsb"] = pzsb.tile([128, 4096], BF, tag="zsb", name="zsb")
            zsb = st["zsb"]
            zw = [pz.tile([128, 512], FP, tag="z", name="zw") for _ in range(2)]
            for kc9 in range(9):
                if kc9 == 0:
                    lhsT = w1x_sb[:, m * 128:(m + 1) * 128]
                else:
                    j = kc9 - 1
                    lhsT = w1h_sb[:, j * 512 + m * 128: j * 512 + m * 128 + 128]
                for half in range(2):
                    toff = half * 512
                    if kc9 == 0:
                        rhs = st["xs"][0:32, 2 * PAD + toff: 2 * PAD + toff + 512]
                    else:
                        j = kc9 - 1
                        co = j % 2
                        rhs = st["h"][j // 2][:, co * HW + PAD + toff
                                              : co * HW + PAD + toff + 512]
                    nc.tensor.matmul(
                        zw[half][:, :], lhsT, rhs,
                        start=(kc9 == 0), stop=(kc9 == 8),
                    )
            for half in range(2):
                nc.scalar.activation(
                    zsb[:, m * NN + half * 512: m * NN + half * 512 + 512],
                    zw[half][:, :], AF.Relu,
                    scale=bns_sb[:, m: m + 1], bias=bnt_sb[:, m: m + 1],
                )

        def head_finish(st):
            s = st["s"]
            zsb = st["zsb"]
            y_s = pfin.tile([1, NN], FP, tag="ys")
            for c2 in range(2):
                yp = pvar.tile([1, 512], FP, tag="var", name="yp")
                for m in range(4):
                    nc.tensor.matmul(
                        yp[0:1, :], w2_sb[:, m: m + 1],
                        zsb[:, m * NN + c2 * 512: m * NN + c2 * 512 + 512],
                        start=(m == 0), stop=(m == 3),
                    )
                nc.vector.tensor_copy(y_s[:, c2 * 512:(c2 + 1) * 512], yp[0:1, :])
            if b2_val != 0.0:
                nc.scalar.add(y_s[:], y_s[:], b2_val)
            m_s = pfin.tile([1, NN], FP, tag="ms")
            nc.sync.dma_start(m_s[:], obs[s: s + 1, 0:NN])
            yf = pfin.tile([1, NN], FP, tag="yfin")
            nc.gpsimd.memset(yf[:], MIN_VAL)
            nc.vector.copy_predicated(yf[:], m_s[:].bitcast(mybir.dt.uint32), y_s[:])
            nc.sync.dma_start(y_out[s: s + 1, :], yf[:])

        def head_units(st):
            return [lambda m=m, st=st: w1_chunk(st, m) for m in range(4)] \
                + [lambda st=st: head_finish(st)]

        headq = []

        def flush(k=None):
            n = len(headq) if k is None else min(k, len(headq))
            for _ in range(n):
                headq.pop(0)()

        prev = [prep_x(0), prep_x(1)]
        for l in range(4):
            layer_round(prev, l)
        for p in range(1, 4):
            for st in prev:
                headq.extend(head_units(st))
            cur = [prep_x(2 * p), prep_x(2 * p + 1)]
            for l in range(4):
                layer_round(cur, l)
                flush(3)
            flush()
            prev = cur
        for st in prev:
            headq.extend(head_units(st))
        flush()

    nc.finalize()
    return nc


_BUILD_CACHE = {}
_CFG = None


def _get_nc(has_gin_bias=None, b2_val=None, use_bf16=None) -> bass.Bass:
    cfg = _CFG
    key = (cfg["has_gin_bias"], cfg["fast_ln"], cfg["gscal"], cfg["b2_val"])
    if key not in _BUILD_CACHE:
        _BUILD_CACHE[key] = _build(cfg)
    return _BUILD_CACHE[key]


def prep_maps(observations, W0, b0, g0, be0, Ws, bs, gs, bes,
              W1, b1, bn_g, bn_b, bn_m, bn_v, W2, b2, **_ignored):
    global _CFG
    obs = np.ascontiguousarray(np.asarray(observations, np.float32))
    W0 = np.asarray(W0, np.float32)
    Ws = np.asarray(Ws, np.float32)
    W1 = np.asarray(W1, np.float32)
    W2 = np.asarray(W2, np.float32)

    W0c = np.ascontiguousarray(W0 - W0.mean(axis=1, keepdims=True))
    Wsc = np.ascontiguousarray(Ws - Ws.mean(axis=2, keepdims=True))

    gg = np.stack([np.asarray(g0, np.float32)]
                  + [np.asarray(gs, np.float32)[i] for i in range(3)])
    bb = np.stack([np.asarray(be0, np.float32)]
                  + [np.asarray(bes, np.float32)[i] for i in range(3)])
    graw = np.stack([np.asarray(b0, np.float32)]
                    + [np.asarray(bs, np.float32)[i] for i in range(3)])
    gbias = np.ascontiguousarray(graw - graw.mean(axis=1, keepdims=True))
    has_gin_bias = bool(np.any(np.abs(gbias) > 0.0))

    g_const = all(np.all(gg[i] == gg[i][0]) for i in range(4))
    fast_ln = bool(np.all(bb == 0.0) and g_const
                   and all(gg[i][0] > 0 for i in range(4)))
    gscal = tuple(float(gg[i][0]) for i in range(4)) if fast_ln else (0.0,) * 4

    bn_scale = (np.asarray(bn_g, np.float32)
                / np.sqrt(np.asarray(bn_v, np.float32) + EPS_BN)).astype(np.float32)
    bn_shift = ((np.asarray(b1, np.float32) - np.asarray(bn_m, np.float32)) * bn_scale
                + np.asarray(bn_b, np.float32)).astype(np.float32)
    b2_val = float(np.asarray(b2, np.float32).reshape(-1)[0])

    def bf16(a):
        t = np.ascontiguousarray(a, np.float32).view(np.uint32)
        r = ((t + 0x7FFF + ((t >> 16) & 1)) >> 16).astype(np.uint16)
        return r

    w1x = np.ascontiguousarray(W1[:F_IN])
    w1h = np.ascontiguousarray(W1[F_IN:].reshape(8, 128, 512))
    w2r = np.ascontiguousarray(W2.reshape(4, 128))

    _CFG = {"has_gin_bias": has_gin_bias, "fast_ln": fast_ln,
            "gscal": gscal, "b2_val": b2_val}

    shared = {
        "w0": bf16(W0c), "ws": bf16(Wsc), "w1x": bf16(w1x),
        "w1h": bf16(w1h), "w2": bf16(w2r),
        "bns": bn_scale, "bnt": bn_shift,
    }
    if has_gin_bias:
        shared["gbias"] = bf16(gbias)
    if not fast_ln:
        shared["grows"] = np.ascontiguousarray(gg)
        shared["bbr"] = np.ascontiguousarray(bb)
    in_maps = []
    for c in range(NCORE):
        m = dict(shared)
        m["obs"] = np.ascontiguousarray(obs[c * S: (c + 1) * S])
        in_maps.append(m)
    return in_maps, has_gin_bias, b2_val


def kernel(**inputs) -> np.ndarray:
    global LAST_EXEC_NS
    in_maps, has_gin_bias, b2_val = prep_maps(**inputs)
    nc = _get_nc(has_gin_bias, b2_val, USE_BF16)
    res = run_bass_kernel_spmd(
        nc, in_maps, list(range(NCORE)), trace=PROFILE, **TRACE_KWARGS
    )
    LAST_EXEC_NS = res.exec_time_ns
    y = np.concatenate([res.results[c]["y"] for c in range(NCORE)], axis=0)
    return y.reshape(B, NN).astype(np.float32)
